# revision 1
# baseline (speedup 1.0000x reference)
"""Trainium2 Bass kernel for 16-head MHA with RoPE (B=1, S=4096, D=1024).

Sharding: tensor-parallel over heads — 2 heads per core on 8 cores.
Per-core pipeline (all matmuls bf16, fp32 PSUM accumulation):
  1. Load hidden transposed [d, s] (host-prepared bf16) + weight slices.
  2. Projections: q_T/k_T/v_T [c=128, s=4096] with weight chunks stationary.
  3. RoPE on q_T/k_T in fp32 via partition-swap trick (channels permuted
     host-side to [evens | odds] per head so rotation pairs sit 32 apart).
  4. v_T -> DMA-transpose -> v_nat [s, c]; scaled by f[k]=exp(mask_add[k])
     (exact mask handling folded into V and the denominator vector).
  5. Attention per q-tile of 512: scores computed TRANSPOSED S_T[k, q] so
     softmax needs no vector-engine reductions: exp on ScalarE
     (scale=1/8 folded in), denominator = f^T @ P_T on the PE,
     ctx_T accumulated over 32 k-chunks in PSUM (2 heads col-tiled).
  6. Reciprocal of denominators on DVE, broadcast across partitions via a
     rank-1 PE matmul, single fused normalize+cast to bf16.
  7. Out-projection with ctx_T stationary; fp32 partial written to DRAM.
Host sums the 8 partials.
"""

import functools

import numpy as np
import ml_dtypes

import concourse.bass as bass
import concourse.tile as tile
import concourse.mybir as mybir
from concourse.bass_utils import run_bass_kernel_spmd

BF16 = mybir.dt.bfloat16
F32 = mybir.dt.float32
F32R = mybir.dt.float32r
bf16 = ml_dtypes.bfloat16

S = 4096      # sequence length
D = 1024      # model dim
HD = 64       # head dim
C = 128       # channels per core (2 heads)
NDC = 8       # contraction chunks of 128 over D
NKC = 32      # key chunks of 128 over S
NQT = 8       # query tiles of 512
QT = 512
GRP = 3       # k-chunks per exp group (3 chunks -> 6 PSUM banks? no: 2 banks/chunk)


_NO_SPLIT = (
    mybir.InstEventSemaphore,
    mybir.InstUnconditionalBranch,
)


def _split_multi_waits(nc: bass.Bass) -> None:
    """Hoist extra sem waits onto standalone EventSemaphore carriers.

    This walrus build only supports one sync-wait command per engine
    instruction ("Too many sync wait commands" in setupSyncWait), so any
    instruction Tile scheduled with >1 wait gets all but its last wait moved
    to dedicated InstEventSemaphore instructions placed immediately before it
    in the same engine stream (sequencer blocks on them in program order —
    semantically identical).
    """
    n = 0
    for fn in nc.m.functions:
        for blk in fn.blocks:
            out = []
            for inst in blk.instructions:
                si = inst.sync_info
                if (
                    si is not None
                    and si.on_wait
                    and len(si.on_wait) > 1
                    and not isinstance(inst, _NO_SPLIT)
                    and inst.engine != mybir.EngineType.Unassigned
                ):
                    waits = list(si.on_wait)
                    for w in waits[:-1]:
                        ev = mybir.InstEventSemaphore(name=f"ant_waitsplit_{n}")
                        n += 1
                        ev.engine = inst.engine
                        ev.sync_info = mybir.SyncInfo(on_wait=[w], on_update=[])
                        nc.register_instruction(ev)
                        out.append(ev)
                    si.on_wait = [waits[-1]]
                    inst.sync_info = si
                out.append(inst)
            blk.instructions[:] = out


def build_program() -> bass.Bass:
    nc = bass.Bass()
    hidT_d = nc.declare_dram_parameter("hidT", [D, S], BF16, isOutput=False)
    wq_d = nc.declare_dram_parameter("wq", [128, D], BF16, isOutput=False)
    wk_d = nc.declare_dram_parameter("wk", [128, D], BF16, isOutput=False)
    wv_d = nc.declare_dram_parameter("wv", [128, D], BF16, isOutput=False)
    wo_d = nc.declare_dram_parameter("wo", [128, D], BF16, isOutput=False)
    cos_d = nc.declare_dram_parameter("cosf", [128, S], F32, isOutput=False)
    sin_d = nc.declare_dram_parameter("sinf", [128, S], F32, isOutput=False)
    mask_d = nc.declare_dram_parameter("maskadd", [128, NKC], F32, isOutput=False)
    ones_d = nc.declare_dram_parameter("ones64", [33, 64], F32, isOutput=False)
    out_d = nc.declare_dram_parameter("outp", [S, D], F32, isOutput=True)

    Exp = mybir.ActivationFunctionType.Exp
    mult = mybir.AluOpType.mult
    add = mybir.AluOpType.add

    with tile.TileContext(nc) as tc:
        with (
            tc.tile_pool(name="const", bufs=1) as const,
            tc.tile_pool(name="ppool", bufs=3) as ppool,
        ):
            # ---- persistent SBUF tiles -------------------------------------
            wq_sb = const.tile([128, D], BF16, tag="wq")
            wk_sb = const.tile([128, D], BF16, tag="wk")
            wv_sb = const.tile([128, D], BF16, tag="wv")
            wo_sb = const.tile([128, D], BF16, tag="wo")
            mask_sb = const.tile([128, NKC], F32, tag="mask")
            f_f32 = const.tile([128, NKC], F32, tag="ff32")
            f_bf = const.tile([128, NKC], BF16, tag="fbf")
            ones64 = const.tile([33, 64], F32, tag="ones")
            qT_bf = const.tile([128, S], BF16, tag="qTbf")
            kT_bf = const.tile([128, S], BF16, tag="kTbf")
            v_nat = const.tile([128, S], BF16, tag="vnat")
            ctxn = const.tile([128, S], BF16, tag="ctxn")

            nc.sync.dma_start(out=wq_sb[:], in_=wq_d[:])
            nc.sync.dma_start(out=wk_sb[:], in_=wk_d[:])
            nc.sync.dma_start(out=wv_sb[:], in_=wv_d[:])
            nc.sync.dma_start(out=wo_sb[:], in_=wo_d[:])
            nc.sync.dma_start(out=mask_sb[:], in_=mask_d[:])
            tscratch = const.tile([1, 8], F32, tag="tscratch")
            nc.sync.dma_start(out=ones64[:], in_=ones_d[:])
            # f[k] = exp(mask_add[k]) — also warms the ACT exp table early
            nc.scalar.activation(f_f32[:], mask_sb[:], Exp)
            nc.vector.tensor_copy(f_bf[:], f_f32[:])

            # ---- phase 1: load hidT + projections + rope -------------------
            with (
                tc.tile_pool(name="hid", bufs=1) as hid,
                tc.tile_pool(name="projps", bufs=1, space="PSUM") as projps,
            ):
                hidT_sb = hid.tile([128, NDC * S], BF16, tag="hidT")
                for dc in range(NDC):
                    nc.sync.dma_start(
                        out=hidT_sb[:, dc * S : (dc + 1) * S],
                        in_=hidT_d[dc * 128 : (dc + 1) * 128, :],
                    )
                qT_f32 = hid.tile([128, S], F32, tag="qTf")
                kT_f32 = hid.tile([128, S], F32, tag="kTf")
                vT_bf = hid.tile([128, S], BF16, tag="vTbf")

                def project(w_sb, dst, dst_dtype_cast_only):
                    ps = [projps.tile([128, QT], F32, name=f"pj{st}", tag=f"pj{st}") for st in range(8)]
                    for dc in range(NDC):
                        for st in range(8):
                            nc.tensor.matmul(
                                ps[st][:],
                                lhsT=w_sb[:, dc * 128 : (dc + 1) * 128],
                                rhs=hidT_sb[:, dc * S + st * QT : dc * S + (st + 1) * QT],
                                start=(dc == 0),
                                stop=(dc == NDC - 1),
                            )
                    for st in range(8):
                        nc.vector.tensor_copy(dst[:, st * QT : (st + 1) * QT], ps[st][:])

                project(wq_sb, qT_f32, False)
                project(wk_sb, kT_f32, False)
                project(wv_sb, vT_bf, True)

                # rope streamed in s-segments to bound SBUF: channel rows per
                # head h: [h*64, h*64+32) = even channels ("a"),
                # [h*64+32, h*64+64) = odd ("b");
                # out = x * cos_full + swap(x) * sin_signed
                SEG = S // 2
                with tc.tile_pool(name="ropep", bufs=2) as ropep:
                    for seg in range(2):
                        sc = slice(seg * SEG, (seg + 1) * SEG)
                        cos_sb = ropep.tile([128, SEG], F32, tag="cs")
                        sin_sb = ropep.tile([128, SEG], F32, tag="sn")
                        nc.sync.dma_start(out=cos_sb[:], in_=cos_d[:, sc])
                        nc.sync.dma_start(out=sin_sb[:], in_=sin_d[:, sc])
                        # touch ops absorb the DMA waits on DVE so the rope
                        # tensor_tensor ops stay within the 1-wait TT limit
                        nc.vector.tensor_copy(tscratch[0:1, 0:1], cos_sb[0:1, 0:1])
                        nc.vector.tensor_copy(tscratch[0:1, 1:2], sin_sb[0:1, 0:1])
                        for x_f32, out_bf in ((qT_f32, qT_bf), (kT_f32, kT_bf)):
                            qsw = ropep.tile([128, SEG], F32, tag="qsw", bufs=1)
                            for h in range(2):
                                a = slice(h * 64, h * 64 + 32)
                                b = slice(h * 64 + 32, h * 64 + 64)
                                nc.vector.tensor_copy(qsw[a, :], x_f32[b, sc])
                                nc.vector.tensor_copy(qsw[b, :], x_f32[a, sc])
                            nc.vector.tensor_tensor(
                                x_f32[:, sc], x_f32[:, sc], cos_sb[:], op=mult
                            )
                            nc.vector.tensor_tensor(qsw[:], qsw[:], sin_sb[:], op=mult)
                            nc.vector.tensor_tensor(
                                out_bf[:, sc], x_f32[:, sc], qsw[:], op=add
                            )

                # v_T [c, s] -> v_nat [s, c] stored as 32 chunks [128, 128]
                nc.sync.dma_start_transpose(
                    out=v_nat[:].rearrange("p (kc c) -> p kc c", kc=NKC),
                    in_=vT_bf[:],
                )
                # fold mask factor f[k] into V rows (and later the denominator)
                for kc in range(NKC):
                    nc.vector.tensor_scalar(
                        v_nat[:, kc * 128 : (kc + 1) * 128],
                        v_nat[:, kc * 128 : (kc + 1) * 128],
                        f_f32[:, kc : kc + 1],
                        None,
                        op0=mult,
                    )

            # ---- phase 2: attention ---------------------------------------
            with (
                tc.tile_pool(name="sgps", bufs=2, space="PSUM") as sgps,
                tc.tile_pool(name="ctxps", bufs=1, space="PSUM") as ctxps,
                tc.tile_pool(name="denps", bufs=1, space="PSUM") as denps,
                tc.tile_pool(name="rpool", bufs=2) as rpool,
            ):
                for qt in range(NQT):
                    qc = slice(qt * QT, (qt + 1) * QT)
                    ctx_ps = ctxps.tile([128, QT], F32, tag="ctx")
                    den_ps = denps.tile([128, QT], F32, tag="den")
                    # 64 (chunk, head) score tiles per q-tile, processed in
                    # groups of GRP PSUM banks (double-buffered: 2*GRP banks)
                    slots = [(c, h) for c in range(NKC) for h in range(2)]
                    for g0 in range(0, len(slots), GRP):
                        grp = slots[g0 : g0 + GRP]
                        nb = len(grp)
                        sg = sgps.tile([128, GRP * QT], F32, tag="sg")
                        Pt = ppool.tile([128, GRP * QT], BF16, tag="pt")
                        for i, (c, h) in enumerate(grp):
                            hr = slice(h * 64, (h + 1) * 64)
                            nc.tensor.matmul(
                                sg[:, i * QT : (i + 1) * QT],
                                lhsT=kT_bf[hr, c * 128 : (c + 1) * 128],
                                rhs=qT_bf[hr, qc],
                                start=True,
                                stop=True,
                            )
                        nc.scalar.activation(
                            Pt[:, : nb * QT], sg[:, : nb * QT], Exp, scale=0.125
                        )
                        for i, (c, h) in enumerate(grp):
                            Ps = Pt[:, i * QT : (i + 1) * QT]
                            vcol = c * 128 + h * 64
                            nc.tensor.matmul(
                                ctx_ps[h * 64 : (h + 1) * 64, :],
                                lhsT=v_nat[:, vcol : vcol + 64],
                                rhs=Ps,
                                start=(c == 0),
                                stop=(c == NKC - 1),
                            )
                            nc.tensor.matmul(
                                den_ps[32 * h : 32 * h + 1, :],
                                lhsT=f_bf[:, c : c + 1],
                                rhs=Ps,
                                start=(c == 0),
                                stop=(c == NKC - 1),
                            )
                    recip = rpool.tile([33, QT], F32, tag="recip")
                    # touches: absorb the PE wait (den_ps) and the slot-reuse
                    # WAR waits (recip) ahead of the wait-slot-limited
                    # Reciprocal instructions
                    nc.vector.tensor_copy(tscratch[0:1, 2:3], den_ps[0:1, 0:1])
                    nc.vector.tensor_copy(recip[0:1, 0:1], tscratch[0:1, 2:3])
                    nc.vector.reciprocal(recip[0:1, :], den_ps[0:1, :])
                    nc.vector.reciprocal(recip[32:33, :], den_ps[32:33, :])
                    # broadcast recip across 64 partitions per head via PE
                    nc.tensor.matmul(
                        den_ps[0:64, :],
                        lhsT=ones64[0:1, :],
                        rhs=recip[0:1, :],
                        start=True,
                        stop=True,
                    )
                    nc.tensor.matmul(
                        den_ps[64:128, :],
                        lhsT=ones64[32:33, :],
                        rhs=recip[32:33, :],
                        start=True,
                        stop=True,
                    )
                    recb_sb = rpool.tile([128, QT], F32, tag="recb")
                    nc.vector.tensor_copy(recb_sb[:], den_ps[:])
                    nc.vector.tensor_tensor(
                        ctxn[:, qc], ctx_ps[:], recb_sb[:], op=mult
                    )

            # ---- phase 3: output projection -------------------------------
            with (
                tc.tile_pool(name="ops", bufs=3, space="PSUM") as ops_pool,
                tc.tile_pool(name="outsb", bufs=3) as outsb_pool,
            ):
                for i in range(32):
                    ops_ = ops_pool.tile([128, D], F32, tag="ops")
                    for j in range(2):
                        nc.tensor.matmul(
                            ops_[:, j * QT : (j + 1) * QT],
                            lhsT=ctxn[:, i * 128 : (i + 1) * 128],
                            rhs=wo_sb[:, j * QT : (j + 1) * QT],
                            start=True,
                            stop=True,
                        )
                    osb = outsb_pool.tile([128, D], F32, tag="osb")
                    nc.vector.tensor_copy(osb[:], ops_[:])
                    nc.sync.dma_start(
                        out=out_d[i * 128 : (i + 1) * 128, :], in_=osb[:]
                    )

    _split_multi_waits(nc)
    return nc


@functools.cache
def _cached_program() -> bass.Bass:
    return build_program()


def _prep_inputs(hidden_states, freqs_cis, attention_mask, wq, wk, wv, wo):
    hid = np.asarray(hidden_states, np.float32).reshape(S, D)
    hidT = np.ascontiguousarray(hid.T).astype(bf16)

    # within-head channel permutation: evens then odds (rope pairs 32 apart)
    perm1 = np.concatenate([np.arange(0, HD, 2), np.arange(1, HD, 2)])
    perm = np.concatenate([perm1, perm1 + HD])  # for the 2 heads of a core

    fc = np.asarray(freqs_cis, np.float32)
    cosT = np.ascontiguousarray(fc[:, :, 0].T)  # [32, S]
    sinT = np.ascontiguousarray(fc[:, :, 1].T)
    cosf = np.concatenate([cosT, cosT, cosT, cosT], 0).astype(np.float32)
    sinf = np.concatenate([-sinT, sinT, -sinT, sinT], 0).astype(np.float32)

    mask_add = (1.0 - np.asarray(attention_mask, np.float32).reshape(S)) * -10000.0
    maskadd = np.ascontiguousarray(mask_add.reshape(NKC, 128).T).astype(np.float32)

    def wlayout(w):  # [1024, 128] -> [128 partitions, chunk-major 1024]
        w = np.ascontiguousarray(w)
        return np.ascontiguousarray(
            w.reshape(NDC, 128, 128).transpose(1, 0, 2).reshape(128, D)
        ).astype(bf16)

    in_maps = []
    for core in range(8):
        cols = slice(core * 128, (core + 1) * 128)
        in_maps.append(
            {
                "hidT": hidT,
                "wq": wlayout(np.asarray(wq, np.float32)[:, cols][:, perm]),
                "wk": wlayout(np.asarray(wk, np.float32)[:, cols][:, perm]),
                "wv": wlayout(np.asarray(wv, np.float32)[:, cols]),
                "wo": np.ascontiguousarray(np.asarray(wo, np.float32)[cols, :]).astype(bf16),
                "cosf": cosf,
                "sinf": sinf,
                "maskadd": maskadd,
                "ones64": np.ones((33, 64), np.float32),
            }
        )
    return in_maps


def run_sharded(in_maps, **kwargs):
    nc = _cached_program()
    return run_bass_kernel_spmd(nc, in_maps, list(range(8)), **kwargs)


def kernel(hidden_states, freqs_cis, attention_mask, wq, wk, wv, wo):
    in_maps = _prep_inputs(
        hidden_states, freqs_cis, attention_mask, wq, wk, wv, wo
    )
    res = run_sharded(in_maps).results
    out = np.zeros((S, D), np.float32)
    for r in res:
        out += np.asarray(r["outp"], np.float32)
    return out.reshape(1, S, D)


if __name__ == "__main__":
    import reference

    inputs = reference.setup_inputs()
    inputs = {k: np.asarray(v) for k, v in inputs.items()}
    expected = np.asarray(reference.reference(**inputs))
    actual = kernel(**inputs)
    err = np.abs(actual - expected).max() / np.abs(expected).max()
    print("Relative error:", err)



# revision 10
# speedup vs baseline: 1.5853x; 1.5853x over previous
"""Trainium2 Bass kernel for 16-head MHA with RoPE (B=1, S=4096, D=1024).

Sharding: tensor-parallel over heads — 2 heads per core on 8 cores.
Per-core pipeline (all matmuls bf16, fp32 PSUM accumulation):
  1. Load hidden transposed [d, s] (host-prepared bf16) + weight slices.
  2. Projections in order Q, V, K; per projection the weight chunks are
     stationary and hidT streams; PSUM copied straight to bf16 SBUF.
  3. RoPE on q/k in bf16 via the partition-swap trick (channels permuted
     host-side to [evens | odds] per head). K is roped in 512-col segments
     so attention can start as soon as the first segments land.
  4. V transposed (xbar DMA) to v_nat [s, c], scaled by f[k]=exp(mask_add),
     then restrided into 66-col blocks per (chunk, head) with f appended as
     a 65th stationary column — the ctx matmul then accumulates the softmax
     denominator into PSUM partition 64 for free (no separate den matmuls).
  5. Attention per q-tile of 512, transposed scores S_T[k, q]: exp on
     ScalarE (scale=1/8 folded), ctx+den accumulate over 32 k-chunks into
     two [65, 512] PSUM banks (one per head). Scores double-buffered with
     one-group lookahead so the PE never stalls on the exp.
  6. Denominator reciprocals via reciprocal_approx_fast straight from PSUM,
     broadcast across partitions with a rank-1 f32r PE matmul into a
     borrowed score-ring bank; fused normalize+cast to bf16.
  7. Out-projection of q-tile t interleaved into q-tile t+1 through the
     same PSUM ring; partials written to DRAM as fp16.
Host sums the 8 fp16 partials in fp32.
"""

import functools

import numpy as np
import ml_dtypes

import concourse.bass as bass
import concourse.tile as tile
import concourse.mybir as mybir
from concourse.bass_utils import run_bass_kernel_spmd

BF16 = mybir.dt.bfloat16
F16 = mybir.dt.float16
F32 = mybir.dt.float32
F32R = mybir.dt.float32r
bf16 = ml_dtypes.bfloat16

S = 4096      # sequence length
D = 1024      # model dim
HD = 64       # head dim
C = 128       # channels per core (2 heads)
NDC = 8       # contraction chunks of 128 over D
NKC = 32      # key chunks of 128 over S
NQT = 8       # query tiles of 512
QT = 512
GRP = 3       # score (chunk, head) slots per exp group / PSUM bank triple
VB = 66       # v4 block stride per (chunk, head): 64 V cols + f col + pad


_NO_SPLIT = (
    mybir.InstEventSemaphore,
    mybir.InstUnconditionalBranch,
)


def _split_multi_waits(nc: bass.Bass) -> None:
    """Hoist extra sem waits onto standalone EventSemaphore carriers.

    This walrus build only supports one sync-wait command per engine
    instruction ("Too many sync wait commands" in setupSyncWait), so any
    instruction Tile scheduled with >1 wait gets all but its last wait moved
    to dedicated InstEventSemaphore instructions placed immediately before it
    in the same engine stream (sequencer blocks on them in program order —
    semantically identical).
    """
    n = 0
    for fn in nc.m.functions:
        for blk in fn.blocks:
            out = []
            for inst in blk.instructions:
                si = inst.sync_info
                if (
                    si is not None
                    and si.on_wait
                    and len(si.on_wait) > 1
                    and not isinstance(inst, _NO_SPLIT)
                    and inst.engine != mybir.EngineType.Unassigned
                ):
                    waits = list(si.on_wait)
                    for w in waits[:-1]:
                        ev = mybir.InstEventSemaphore(name=f"ant_waitsplit_{n}")
                        n += 1
                        ev.engine = inst.engine
                        ev.sync_info = mybir.SyncInfo(on_wait=[w], on_update=[])
                        nc.register_instruction(ev)
                        out.append(ev)
                    si.on_wait = [waits[-1]]
                    inst.sync_info = si
                out.append(inst)
            blk.instructions[:] = out


def build_program() -> bass.Bass:
    nc = bass.Bass()
    hidT_d = nc.declare_dram_parameter("hidT", [D, S], BF16, isOutput=False)
    wq_d = nc.declare_dram_parameter("wq", [128, D], BF16, isOutput=False)
    wk_d = nc.declare_dram_parameter("wk", [128, D], BF16, isOutput=False)
    wv_d = nc.declare_dram_parameter("wv", [128, D], BF16, isOutput=False)
    wo_d = nc.declare_dram_parameter("wo", [128, D], BF16, isOutput=False)
    cos_d = nc.declare_dram_parameter("cosf", [128, S], BF16, isOutput=False)
    sin_d = nc.declare_dram_parameter("sinf", [128, S], BF16, isOutput=False)
    mask_d = nc.declare_dram_parameter("maskadd", [128, NKC], F32, isOutput=False)
    ones_d = nc.declare_dram_parameter("ones64", [1, 64], F32, isOutput=False)
    out_d = nc.declare_dram_parameter("outp", [S, D], F16, isOutput=True)

    Exp = mybir.ActivationFunctionType.Exp
    mult = mybir.AluOpType.mult
    add = mybir.AluOpType.add

    with tile.TileContext(nc) as tc:
        with (
            tc.tile_pool(name="const", bufs=1) as const,
            tc.tile_pool(name="ppool", bufs=3) as ppool,
        ):
            # ---- persistent SBUF tiles -------------------------------------
            wq_sb = const.tile([128, D], BF16, tag="wq")
            wk_sb = const.tile([128, D], BF16, tag="wk")
            wv_sb = const.tile([128, D], BF16, tag="wv")
            wo_sb = const.tile([128, D], BF16, tag="wo")
            mask_sb = const.tile([128, NKC], F32, tag="mask")
            f_f32 = const.tile([128, NKC], F32, tag="ff32")
            ones_sb = const.tile([1, 64], F32, tag="ones")
            cos_sb = const.tile([128, S], BF16, tag="cosf")
            sin_sb = const.tile([128, S], BF16, tag="sinf")
            qT_bf = const.tile([128, S], BF16, tag="qTbf")
            kT_bf = const.tile([128, S], BF16, tag="kTbf")
            qsw = const.tile([128, S], BF16, tag="qsw")
            v4 = const.tile([128, NKC * 2 * VB], BF16, tag="v4")
            ctxn = const.tile([128, S], BF16, tag="ctxn")

            nc.sync.dma_start(out=wq_sb[:], in_=wq_d[:])
            nc.sync.dma_start(out=wv_sb[:], in_=wv_d[:])
            nc.sync.dma_start(out=wk_sb[:], in_=wk_d[:])
            nc.sync.dma_start(out=wo_sb[:], in_=wo_d[:])
            nc.sync.dma_start(out=mask_sb[:], in_=mask_d[:])
            nc.sync.dma_start(out=ones_sb[:], in_=ones_d[:])
            nc.sync.dma_start(out=cos_sb[:], in_=cos_d[:])
            nc.sync.dma_start(out=sin_sb[:], in_=sin_d[:])
            # f[k] = exp(mask_add[k]) — also warms the ACT exp table early
            nc.scalar.activation(f_f32[:], mask_sb[:], Exp)

            # ---- phase 1: load hidT + projections + rope -------------------
            with (
                tc.tile_pool(name="hid", bufs=1) as hid,
                tc.tile_pool(name="projps", bufs=1, space="PSUM") as projps,
            ):
                hidT_sb = hid.tile([128, NDC * S], BF16, tag="hidT")
                for dc in range(NDC):
                    nc.sync.dma_start(
                        out=hidT_sb[:, dc * S : (dc + 1) * S],
                        in_=hidT_d[dc * 128 : (dc + 1) * 128, :],
                    )
                vT_bf = hid.tile([128, S], BF16, tag="vTbf")
                v_nat = hid.tile([128, S], BF16, tag="vnat")

                ps = [
                    projps.tile([128, QT], F32, name=f"pj{st}", tag=f"pj{st}")
                    for st in range(8)
                ]

                def project(w_sb, dst):
                    for dc in range(NDC):
                        for st in range(8):
                            nc.tensor.matmul(
                                ps[st][:],
                                lhsT=w_sb[:, dc * 128 : (dc + 1) * 128],
                                rhs=hidT_sb[:, dc * S + st * QT : dc * S + (st + 1) * QT],
                                start=(dc == 0),
                                stop=(dc == NDC - 1),
                            )
                    for st in range(8):
                        nc.vector.tensor_copy(dst[:, st * QT : (st + 1) * QT], ps[st][:])

                def rope(x_bf, s0, s1):
                    # channel rows per head h: [h*64, h*64+32) = evens ("a"),
                    # [h*64+32, h*64+64) = odds ("b");
                    # out = x * cos_full + swap(x) * sin_signed
                    sc = slice(s0, s1)
                    for h in range(2):
                        a = slice(h * 64, h * 64 + 32)
                        b = slice(h * 64 + 32, h * 64 + 64)
                        nc.vector.tensor_copy(qsw[a, sc], x_bf[b, sc])
                        nc.vector.tensor_copy(qsw[b, sc], x_bf[a, sc])
                    nc.vector.tensor_tensor(x_bf[:, sc], x_bf[:, sc], cos_sb[:, sc], op=mult)
                    nc.vector.tensor_tensor(qsw[:, sc], qsw[:, sc], sin_sb[:, sc], op=mult)
                    nc.vector.tensor_tensor(x_bf[:, sc], x_bf[:, sc], qsw[:, sc], op=add)

                # Q projection + full-width rope
                project(wq_sb, qT_bf)
                rope(qT_bf, 0, S)

                # V projection + transpose + f-scale + restride into v4
                project(wv_sb, vT_bf)
                nc.sync.dma_start_transpose(
                    out=v_nat[:].rearrange("p (kc c) -> p kc c", kc=NKC),
                    in_=vT_bf[:],
                )
                for kc in range(NKC):
                    nc.vector.tensor_scalar(
                        v_nat[:, kc * 128 : (kc + 1) * 128],
                        v_nat[:, kc * 128 : (kc + 1) * 128],
                        f_f32[:, kc : kc + 1],
                        None,
                        op0=mult,
                    )
                v4r = v4[:].rearrange("p (kc h c) -> p kc h c", kc=NKC, h=2)
                vnr = v_nat[:].rearrange("p (kc h c) -> p kc h c", kc=NKC, h=2)
                nc.vector.tensor_copy(v4r[:, :, :, 0:64], vnr[:, :, :, :])
                for h in range(2):
                    nc.vector.tensor_copy(
                        v4r[:, :, h : h + 1, 64:65],
                        f_f32[:].unsqueeze(-1).unsqueeze(-1),
                    )

                # K projection + rope in 512-col segments (feeds attention
                # chunk-by-chunk)
                project(wk_sb, kT_bf)
                for seg in range(8):
                    rope(kT_bf, seg * QT, (seg + 1) * QT)

            # ---- phase 2: attention + fused out-projection -----------------
            slots = [(c, h) for c in range(NKC) for h in range(2)]
            groups = [slots[i : i + GRP] for i in range(0, len(slots), GRP)]
            NG = len(groups)  # 22

            with (
                tc.tile_pool(name="sgps", bufs=2, space="PSUM") as sgps,
                tc.tile_pool(name="ctxps", bufs=1, space="PSUM") as ctxps,
                tc.tile_pool(name="rpool", bufs=2) as rpool,
                tc.tile_pool(name="opool", bufs=3) as opool,
            ):
                ctxA = ctxps.tile([65, QT], F32, tag="ctxA")
                ctxB = ctxps.tile([65, QT], F32, tag="ctxB")
                ctx_banks = (ctxA, ctxB)

                def emit_scores(qt, g):
                    qc = slice(qt * QT, (qt + 1) * QT)
                    grp = groups[g]
                    nb = len(grp)
                    sg = sgps.tile([128, GRP * QT], F32, tag="sg")
                    Pt = ppool.tile([128, GRP * QT], BF16, tag="pt")
                    for i, (c, h) in enumerate(grp):
                        hr = slice(h * 64, (h + 1) * 64)
                        nc.tensor.matmul(
                            sg[:, i * QT : (i + 1) * QT],
                            lhsT=kT_bf[hr, c * 128 : (c + 1) * 128],
                            rhs=qT_bf[hr, qc],
                            start=True,
                            stop=True,
                        )
                    nc.scalar.activation(
                        Pt[:, : nb * QT], sg[:, : nb * QT], Exp, scale=0.125
                    )
                    return Pt

                def emit_ctx(g, Pt):
                    grp = groups[g]
                    for i, (c, h) in enumerate(grp):
                        vcol = (c * 2 + h) * VB
                        nc.tensor.matmul(
                            ctx_banks[h][:, :],
                            lhsT=v4[:, vcol : vcol + 65],
                            rhs=Pt[:, i * QT : (i + 1) * QT],
                            start=(c == 0),
                            stop=(c == NKC - 1),
                        )

                def emit_recips(qt):
                    # Evacuate the ctx banks to SBUF right after the final ctx
                    # accumulation (frees the banks for the next qtile fast),
                    # then the reciprocal of the two denominator rows runs off
                    # the PSUM-WAR critical path.
                    dsb = rpool.tile([33, QT], F32, tag="dsb")
                    nc.vector.tensor_copy(dsb[0:1, :], ctxA[64:65, :])
                    nc.vector.tensor_copy(dsb[32:33, :], ctxB[64:65, :])
                    ctxc = rpool.tile([128, QT], F32, tag="ctxc")
                    nc.vector.tensor_copy(ctxc[0:64, :], ctxA[0:64, :])
                    nc.vector.tensor_copy(ctxc[64:128, :], ctxB[0:64, :])
                    rcp = rpool.tile([33, QT], F32, tag="rcp")
                    nc.vector.reciprocal(rcp[0:1, :], dsb[0:1, :])
                    nc.vector.reciprocal(rcp[32:33, :], dsb[32:33, :])
                    rsb = rpool.tile([1, 2 * QT], F32, tag="rsb")
                    nc.vector.tensor_copy(rsb[0:1, 0:QT], rcp[0:1, :])
                    nc.vector.tensor_copy(rsb[0:1, QT : 2 * QT], rcp[32:33, :])
                    return rsb, ctxc

                def emit_normalize(qt, rsb, ctxc):
                    # broadcast recips across partitions via rank-1 PE matmuls
                    # into a borrowed score-ring bank, then fused
                    # normalize+cast into ctxn (all SBUF inputs — no ctx-bank
                    # dependence left).
                    qc = slice(qt * QT, (qt + 1) * QT)
                    bt = sgps.tile([128, GRP * QT], F32, tag="sg")
                    nc.tensor.matmul(
                        bt[0:64, 0:QT],
                        lhsT=ones_sb[:],
                        rhs=rsb[0:1, 0:QT],
                        start=True,
                        stop=True,
                    )
                    nc.tensor.matmul(
                        bt[64:128, 0:QT],
                        lhsT=ones_sb[:],
                        rhs=rsb[0:1, QT : 2 * QT],
                        start=True,
                        stop=True,
                        tile_position=(0, 64),
                    )
                    recb = rpool.tile([128, QT], F32, tag="recb")
                    nc.vector.tensor_copy(recb[:], bt[:, 0:QT])
                    nc.vector.tensor_tensor(
                        ctxn[0:64, qc], ctxc[0:64, :], recb[0:64, :], op=mult
                    )
                    nc.vector.tensor_tensor(
                        ctxn[64:128, qc], ctxc[64:128, :], recb[64:128, :], op=mult
                    )

                def emit_outproj(qt, j):
                    # out rows [qt*512 + j*128, +128) = ctxn_chunk^T @ wo
                    ot = sgps.tile([128, GRP * QT], F32, tag="sg")
                    col = qt * QT + j * 128
                    for half in range(2):
                        nc.tensor.matmul(
                            ot[:, half * QT : (half + 1) * QT],
                            lhsT=ctxn[:, col : col + 128],
                            rhs=wo_sb[:, half * QT : (half + 1) * QT],
                            start=True,
                            stop=True,
                        )
                    osb = opool.tile([128, D], F16, tag="osb")
                    nc.vector.tensor_copy(osb[:], ot[:, 0:D])
                    nc.sync.dma_start(
                        out=out_d[col : col + 128, :], in_=osb[:]
                    )

                norm_prev = None
                for qt in range(NQT):
                    pts = {}
                    for g in range(NG):
                        pts[g] = emit_scores(qt, g)
                        if g == 4 and qt > 0:
                            emit_normalize(qt - 1, *norm_prev)
                        if qt > 0 and g in (7, 12, 16, 21):
                            emit_outproj(qt - 1, {7: 0, 12: 1, 16: 2, 21: 3}[g])
                        if g >= 1:
                            emit_ctx(g - 1, pts.pop(g - 1))
                    emit_ctx(NG - 1, pts.pop(NG - 1))
                    norm_prev = emit_recips(qt)
                emit_normalize(NQT - 1, *norm_prev)
                for j in range(4):
                    emit_outproj(NQT - 1, j)

    _split_multi_waits(nc)
    return nc


@functools.cache
def _cached_program() -> bass.Bass:
    return build_program()


def _prep_inputs(hidden_states, freqs_cis, attention_mask, wq, wk, wv, wo):
    hid = np.asarray(hidden_states, np.float32).reshape(S, D)
    hidT = np.ascontiguousarray(hid.T).astype(bf16)

    # within-head channel permutation: evens then odds (rope pairs 32 apart)
    perm1 = np.concatenate([np.arange(0, HD, 2), np.arange(1, HD, 2)])
    perm = np.concatenate([perm1, perm1 + HD])  # for the 2 heads of a core

    fc = np.asarray(freqs_cis, np.float32)
    cosT = np.ascontiguousarray(fc[:, :, 0].T)  # [32, S]
    sinT = np.ascontiguousarray(fc[:, :, 1].T)
    cosf = np.concatenate([cosT, cosT, cosT, cosT], 0).astype(bf16)
    sinf = np.concatenate([-sinT, sinT, -sinT, sinT], 0).astype(bf16)

    mask_add = (1.0 - np.asarray(attention_mask, np.float32).reshape(S)) * -10000.0
    maskadd = np.ascontiguousarray(mask_add.reshape(NKC, 128).T).astype(np.float32)

    def wlayout(w):  # [1024, 128] -> [128 partitions, chunk-major 1024]
        w = np.ascontiguousarray(w)
        return np.ascontiguousarray(
            w.reshape(NDC, 128, 128).transpose(1, 0, 2).reshape(128, D)
        ).astype(bf16)

    in_maps = []
    for core in range(8):
        cols = slice(core * 128, (core + 1) * 128)
        in_maps.append(
            {
                "hidT": hidT,
                "wq": wlayout(np.asarray(wq, np.float32)[:, cols][:, perm]),
                "wk": wlayout(np.asarray(wk, np.float32)[:, cols][:, perm]),
                "wv": wlayout(np.asarray(wv, np.float32)[:, cols]),
                "wo": np.ascontiguousarray(np.asarray(wo, np.float32)[cols, :]).astype(bf16),
                "cosf": cosf,
                "sinf": sinf,
                "maskadd": maskadd,
                "ones64": np.ones((1, 64), np.float32),
            }
        )
    return in_maps


def run_sharded(in_maps, **kwargs):
    nc = _cached_program()
    return run_bass_kernel_spmd(nc, in_maps, list(range(8)), **kwargs)


def kernel(hidden_states, freqs_cis, attention_mask, wq, wk, wv, wo):
    in_maps = _prep_inputs(
        hidden_states, freqs_cis, attention_mask, wq, wk, wv, wo
    )
    res = run_sharded(in_maps).results
    out = np.zeros((S, D), np.float32)
    for r in res:
        out += np.asarray(r["outp"], np.float32)
    return out.reshape(1, S, D)


if __name__ == "__main__":
    import reference

    inputs = reference.setup_inputs()
    inputs = {k: np.asarray(v) for k, v in inputs.items()}
    expected = np.asarray(reference.reference(**inputs))
    actual = kernel(**inputs)
    err = np.abs(actual - expected).max() / np.abs(expected).max()
    print("Relative error:", err)


# revision 14
# speedup vs baseline: 1.7064x; 1.0764x over previous
"""Trainium2 Bass kernel for 16-head MHA with RoPE (B=1, S=4096, D=1024).

Sharding: tensor-parallel over heads — 2 heads per core on 8 cores.
Per-core pipeline (all matmuls bf16, fp32 PSUM accumulation):
  1. Load hidden transposed [d, s] (host-prepared bf16) + weight slices.
  2. Projections in order Q, V, K; per projection the weight chunks are
     stationary and hidT streams; PSUM copied straight to bf16 SBUF.
  3. RoPE on q/k in bf16 via the partition-swap trick (channels permuted
     host-side to [evens | odds] per head). K is roped in 512-col segments
     so attention can start as soon as the first segments land.
  4. V transposed (xbar DMA) to v_nat [s, c], scaled by f[k]=exp(mask_add),
     then restrided into 66-col blocks per (chunk, head) with f appended as
     a 65th stationary column — the ctx matmul then accumulates the softmax
     denominator into PSUM partition 64 for free (no separate den matmuls).
  5. Attention per q-tile of 512, transposed scores S_T[k, q]: exp on
     ScalarE (scale=1/8 folded), ctx+den accumulate over 32 k-chunks into
     two [65, 512] PSUM banks (one per head). Scores double-buffered with
     one-group lookahead so the PE never stalls on the exp.
  6. Denominator reciprocals via reciprocal_approx_fast straight from PSUM,
     broadcast across partitions with a rank-1 f32r PE matmul into a
     borrowed score-ring bank; fused normalize+cast to bf16.
  7. Out-projection of q-tile t interleaved into q-tile t+1 through the
     same PSUM ring; partials written to DRAM as fp16.
Host sums the 8 fp16 partials in fp32.
"""

import functools

import numpy as np
import ml_dtypes

import concourse.bass as bass
import concourse.tile as tile
import concourse.mybir as mybir
from concourse.bass_utils import run_bass_kernel_spmd

BF16 = mybir.dt.bfloat16
F16 = mybir.dt.float16
F32 = mybir.dt.float32
F32R = mybir.dt.float32r
bf16 = ml_dtypes.bfloat16

S = 4096      # sequence length
D = 1024      # model dim
HD = 64       # head dim
C = 128       # channels per core (2 heads)
NDC = 8       # contraction chunks of 128 over D
NKC = 32      # key chunks of 128 over S
NQT = 8       # query tiles of 512
QT = 512
GRP = 3       # score (chunk, head) slots per exp group / PSUM bank triple
VB = 66       # v4 block stride per (chunk, head): 64 V cols + f col + pad


_NO_SPLIT = (
    mybir.InstEventSemaphore,
    mybir.InstUnconditionalBranch,
)


def _split_multi_waits(nc: bass.Bass) -> None:
    """Hoist extra sem waits onto standalone EventSemaphore carriers.

    This walrus build only supports one sync-wait command per engine
    instruction ("Too many sync wait commands" in setupSyncWait), so any
    instruction Tile scheduled with >1 wait gets all but its last wait moved
    to dedicated InstEventSemaphore instructions placed immediately before it
    in the same engine stream (sequencer blocks on them in program order —
    semantically identical).
    """
    n = 0
    for fn in nc.m.functions:
        for blk in fn.blocks:
            out = []
            for inst in blk.instructions:
                si = inst.sync_info
                if (
                    si is not None
                    and si.on_wait
                    and len(si.on_wait) > 1
                    and not isinstance(inst, _NO_SPLIT)
                    and inst.engine != mybir.EngineType.Unassigned
                ):
                    waits = list(si.on_wait)
                    for w in waits[:-1]:
                        ev = mybir.InstEventSemaphore(name=f"ant_waitsplit_{n}")
                        n += 1
                        ev.engine = inst.engine
                        ev.sync_info = mybir.SyncInfo(on_wait=[w], on_update=[])
                        nc.register_instruction(ev)
                        out.append(ev)
                    si.on_wait = [waits[-1]]
                    inst.sync_info = si
                out.append(inst)
            blk.instructions[:] = out


def build_program() -> bass.Bass:
    nc = bass.Bass()
    hidT_d = nc.declare_dram_parameter("hidT", [D, S], BF16, isOutput=False)
    wq_d = nc.declare_dram_parameter("wq", [128, D], BF16, isOutput=False)
    wk_d = nc.declare_dram_parameter("wk", [128, D], BF16, isOutput=False)
    wv_d = nc.declare_dram_parameter("wv", [128, D], BF16, isOutput=False)
    wo_d = nc.declare_dram_parameter("wo", [128, D], BF16, isOutput=False)
    cos_d = nc.declare_dram_parameter("cosf", [128, S], BF16, isOutput=False)
    sin_d = nc.declare_dram_parameter("sinf", [128, S], BF16, isOutput=False)
    mask_d = nc.declare_dram_parameter("maskadd", [128, NKC], F32, isOutput=False)
    ones_d = nc.declare_dram_parameter("ones64", [1, 64], F32, isOutput=False)
    out_d = nc.declare_dram_parameter("outp", [S, D], F16, isOutput=True)

    Exp = mybir.ActivationFunctionType.Exp
    mult = mybir.AluOpType.mult
    add = mybir.AluOpType.add

    with tile.TileContext(nc) as tc:
        with (
            tc.tile_pool(name="const", bufs=1) as const,
            tc.tile_pool(name="ppool", bufs=3) as ppool,
        ):
            # ---- persistent SBUF tiles -------------------------------------
            wq_sb = const.tile([128, D], BF16, tag="wq")
            wk_sb = const.tile([128, D], BF16, tag="wk")
            wv_sb = const.tile([128, D], BF16, tag="wv")
            wo_sb = const.tile([128, D], BF16, tag="wo")
            mask_sb = const.tile([128, NKC], F32, tag="mask")
            f_f32 = const.tile([128, NKC], F32, tag="ff32")
            ones_sb = const.tile([1, 64], F32, tag="ones")
            cos_sb = const.tile([128, S], BF16, tag="cosf")
            sin_sb = const.tile([128, S], BF16, tag="sinf")
            qT_bf = const.tile([128, S], BF16, tag="qTbf")
            kT_bf = const.tile([128, S], BF16, tag="kTbf")
            qsw = const.tile([128, S], BF16, tag="qsw")
            v4 = const.tile([128, NKC * 2 * VB], BF16, tag="v4")
            ctxn = const.tile([128, S], BF16, tag="ctxn")

            nc.sync.dma_start(out=wq_sb[:], in_=wq_d[:])
            nc.sync.dma_start(out=wv_sb[:], in_=wv_d[:])
            nc.sync.dma_start(out=wk_sb[:], in_=wk_d[:])
            nc.sync.dma_start(out=wo_sb[:], in_=wo_d[:])
            nc.sync.dma_start(out=mask_sb[:], in_=mask_d[:])
            nc.sync.dma_start(out=ones_sb[:], in_=ones_d[:])
            nc.sync.dma_start(out=cos_sb[:], in_=cos_d[:])
            nc.sync.dma_start(out=sin_sb[:], in_=sin_d[:])
            # f[k] = exp(mask_add[k]) — also warms the ACT exp table early
            nc.scalar.activation(f_f32[:], mask_sb[:], Exp)

            # ---- phase 1: load hidT + projections + rope -------------------
            with (
                tc.tile_pool(name="hid", bufs=1) as hid,
                tc.tile_pool(name="projps", bufs=1, space="PSUM") as projps,
            ):
                hidT_sb = hid.tile([128, NDC * S], BF16, tag="hidT")
                for dc in range(NDC):
                    nc.sync.dma_start(
                        out=hidT_sb[:, dc * S : (dc + 1) * S],
                        in_=hidT_d[dc * 128 : (dc + 1) * 128, :],
                    )
                vT_bf = hid.tile([128, S], BF16, tag="vTbf")
                v_nat = hid.tile([128, S], BF16, tag="vnat")

                ps = [
                    projps.tile([128, QT], F32, name=f"pj{st}", tag=f"pj{st}")
                    for st in range(8)
                ]

                def project(w_sb, dst, st_outer=False, per_st=None):
                    # dc-outer consumes hidT chunks as they stream in (for the
                    # first, DMA-paced projection); st-outer finishes one
                    # output tile at a time so its PSUM copy (and optional
                    # per-tile continuation like rope) can start immediately.
                    if st_outer:
                        for st in range(8):
                            for dc in range(NDC):
                                nc.tensor.matmul(
                                    ps[st][:],
                                    lhsT=w_sb[:, dc * 128 : (dc + 1) * 128],
                                    rhs=hidT_sb[:, dc * S + st * QT : dc * S + (st + 1) * QT],
                                    start=(dc == 0),
                                    stop=(dc == NDC - 1),
                                )
                            nc.vector.tensor_copy(
                                dst[:, st * QT : (st + 1) * QT], ps[st][:]
                            )
                            if per_st is not None:
                                per_st(st)
                    else:
                        for dc in range(NDC):
                            for st in range(8):
                                nc.tensor.matmul(
                                    ps[st][:],
                                    lhsT=w_sb[:, dc * 128 : (dc + 1) * 128],
                                    rhs=hidT_sb[:, dc * S + st * QT : dc * S + (st + 1) * QT],
                                    start=(dc == 0),
                                    stop=(dc == NDC - 1),
                                )
                        for st in range(8):
                            nc.vector.tensor_copy(
                                dst[:, st * QT : (st + 1) * QT], ps[st][:]
                            )

                def rope(x_bf, s0, s1):
                    # channel rows per head h: [h*64, h*64+32) = evens ("a"),
                    # [h*64+32, h*64+64) = odds ("b");
                    # out = x * cos_full + swap(x) * sin_signed
                    sc = slice(s0, s1)
                    for h in range(2):
                        a = slice(h * 64, h * 64 + 32)
                        b = slice(h * 64 + 32, h * 64 + 64)
                        nc.vector.tensor_copy(qsw[a, sc], x_bf[b, sc])
                        nc.vector.tensor_copy(qsw[b, sc], x_bf[a, sc])
                    nc.vector.tensor_tensor(x_bf[:, sc], x_bf[:, sc], cos_sb[:, sc], op=mult)
                    nc.vector.tensor_tensor(qsw[:, sc], qsw[:, sc], sin_sb[:, sc], op=mult)
                    nc.vector.tensor_tensor(x_bf[:, sc], x_bf[:, sc], qsw[:, sc], op=add)

                # Q projection (dc-outer: paced by the hidT DMA stream) +
                # full-width rope
                project(wq_sb, qT_bf)
                rope(qT_bf, 0, S)

                # V projection (st-outer: copies start early) + transpose +
                # f-scale + restride into v4
                project(wv_sb, vT_bf, st_outer=True)
                nc.sync.dma_start_transpose(
                    out=v_nat[:].rearrange("p (kc c) -> p kc c", kc=NKC),
                    in_=vT_bf[:],
                )
                for kc in range(NKC):
                    nc.vector.tensor_scalar(
                        v_nat[:, kc * 128 : (kc + 1) * 128],
                        v_nat[:, kc * 128 : (kc + 1) * 128],
                        f_f32[:, kc : kc + 1],
                        None,
                        op0=mult,
                    )
                v4r = v4[:].rearrange("p (kc h c) -> p kc h c", kc=NKC, h=2)
                vnr = v_nat[:].rearrange("p (kc h c) -> p kc h c", kc=NKC, h=2)
                nc.vector.tensor_copy(v4r[:, :, :, 0:64], vnr[:, :, :, :])
                for h in range(2):
                    nc.vector.tensor_copy(
                        v4r[:, :, h : h + 1, 64:65],
                        f_f32[:].unsqueeze(-1).unsqueeze(-1),
                    )

                # K projection st-outer with rope fused per 512-col tile —
                # attention scores can start on chunk 0 while later K tiles
                # are still in the PE
                project(
                    wk_sb,
                    kT_bf,
                    st_outer=True,
                    per_st=lambda st: rope(kT_bf, st * QT, (st + 1) * QT),
                )

            # ---- phase 2: attention + fused out-projection -----------------
            slots = [(c, h) for c in range(NKC) for h in range(2)]
            groups = [slots[i : i + GRP] for i in range(0, len(slots), GRP)]
            NG = len(groups)  # 22

            with (
                tc.tile_pool(name="sgps", bufs=2, space="PSUM") as sgps,
                tc.tile_pool(name="ctxps", bufs=1, space="PSUM") as ctxps,
                tc.tile_pool(name="rpool", bufs=2) as rpool,
                tc.tile_pool(name="opool", bufs=3) as opool,
            ):
                ctxA = ctxps.tile([65, QT], F32, tag="ctxA")
                ctxB = ctxps.tile([65, QT], F32, tag="ctxB")
                ctx_banks = (ctxA, ctxB)

                # pre-fill both dsb ring buffers so the single batched
                # reciprocal's untouched rows (1..31) stay finite
                for _ in range(2):
                    dpre = rpool.tile([33, QT], F32, tag="dsb", name="dpre")
                    nc.vector.memset(dpre[:], 1.0)

                def emit_scores(qt, g):
                    qc = slice(qt * QT, (qt + 1) * QT)
                    grp = groups[g]
                    nb = len(grp)
                    sg = sgps.tile([128, GRP * QT], F32, tag="sg")
                    Pt = ppool.tile([128, GRP * QT], BF16, tag="pt")
                    for i, (c, h) in enumerate(grp):
                        hr = slice(h * 64, (h + 1) * 64)
                        nc.tensor.matmul(
                            sg[:, i * QT : (i + 1) * QT],
                            lhsT=kT_bf[hr, c * 128 : (c + 1) * 128],
                            rhs=qT_bf[hr, qc],
                            start=True,
                            stop=True,
                        )
                    nc.scalar.activation(
                        Pt[:, : nb * QT], sg[:, : nb * QT], Exp, scale=0.125
                    )
                    return Pt

                def emit_ctx(g, Pt):
                    grp = groups[g]
                    for i, (c, h) in enumerate(grp):
                        vcol = (c * 2 + h) * VB
                        nc.tensor.matmul(
                            ctx_banks[h][:, :],
                            lhsT=v4[:, vcol : vcol + 65],
                            rhs=Pt[:, i * QT : (i + 1) * QT],
                            start=(c == 0),
                            stop=(c == NKC - 1),
                        )

                def emit_recips(qt):
                    # Evacuate the ctx banks to SBUF right after the final ctx
                    # accumulation (frees the banks for the next qtile fast),
                    # then the reciprocal of the two denominator rows runs off
                    # the PSUM-WAR critical path.
                    dsb = rpool.tile([33, QT], F32, tag="dsb")
                    nc.vector.tensor_copy(dsb[0:1, :], ctxA[64:65, :])
                    nc.vector.tensor_copy(dsb[32:33, :], ctxB[64:65, :])
                    ctxc = rpool.tile([128, QT], F32, tag="ctxc")
                    nc.vector.tensor_copy(ctxc[0:64, :], ctxA[0:64, :])
                    nc.vector.tensor_copy(ctxc[64:128, :], ctxB[0:64, :])
                    rcp = rpool.tile([33, QT], F32, tag="rcp")
                    # one call covers both denominator rows (0 and 32); the
                    # uninitialized rows in between are never read
                    nc.vector.reciprocal(rcp[:, :], dsb[:, :])
                    rsb = rpool.tile([1, 2 * QT], F32, tag="rsb")
                    nc.vector.tensor_copy(rsb[0:1, 0:QT], rcp[0:1, :])
                    nc.vector.tensor_copy(rsb[0:1, QT : 2 * QT], rcp[32:33, :])
                    return rsb, ctxc

                def emit_normalize(qt, rsb, ctxc):
                    # broadcast recips across partitions via rank-1 PE matmuls
                    # into a borrowed score-ring bank, then fused
                    # normalize+cast into ctxn (all SBUF inputs — no ctx-bank
                    # dependence left).
                    qc = slice(qt * QT, (qt + 1) * QT)
                    bt = sgps.tile([128, GRP * QT], F32, tag="sg")
                    nc.tensor.matmul(
                        bt[0:64, 0:QT],
                        lhsT=ones_sb[:],
                        rhs=rsb[0:1, 0:QT],
                        start=True,
                        stop=True,
                    )
                    nc.tensor.matmul(
                        bt[64:128, 0:QT],
                        lhsT=ones_sb[:],
                        rhs=rsb[0:1, QT : 2 * QT],
                        start=True,
                        stop=True,
                        tile_position=(0, 64),
                    )
                    recb = rpool.tile([128, QT], F32, tag="recb")
                    nc.vector.tensor_copy(recb[:], bt[:, 0:QT])
                    nc.vector.tensor_tensor(
                        ctxn[0:64, qc], ctxc[0:64, :], recb[0:64, :], op=mult
                    )
                    nc.vector.tensor_tensor(
                        ctxn[64:128, qc], ctxc[64:128, :], recb[64:128, :], op=mult
                    )

                def emit_outproj(qt, j):
                    # out rows [qt*512 + j*128, +128) = ctxn_chunk^T @ wo
                    ot = sgps.tile([128, GRP * QT], F32, tag="sg")
                    col = qt * QT + j * 128
                    for half in range(2):
                        nc.tensor.matmul(
                            ot[:, half * QT : (half + 1) * QT],
                            lhsT=ctxn[:, col : col + 128],
                            rhs=wo_sb[:, half * QT : (half + 1) * QT],
                            start=True,
                            stop=True,
                        )
                    osb = opool.tile([128, D], F16, tag="osb")
                    nc.vector.tensor_copy(osb[:], ot[:, 0:D])
                    nc.sync.dma_start(
                        out=out_d[col : col + 128, :], in_=osb[:]
                    )

                norm_prev = None
                for qt in range(NQT):
                    pts = {}
                    for g in range(NG):
                        pts[g] = emit_scores(qt, g)
                        if g == 4 and qt > 0:
                            emit_normalize(qt - 1, *norm_prev)
                        if qt > 0 and g in (7, 12, 16, 21):
                            emit_outproj(qt - 1, {7: 0, 12: 1, 16: 2, 21: 3}[g])
                        if g >= 1:
                            emit_ctx(g - 1, pts.pop(g - 1))
                    emit_ctx(NG - 1, pts.pop(NG - 1))
                    norm_prev = emit_recips(qt)
                emit_normalize(NQT - 1, *norm_prev)
                for j in range(4):
                    emit_outproj(NQT - 1, j)

    _split_multi_waits(nc)
    return nc


@functools.cache
def _cached_program() -> bass.Bass:
    return build_program()


def _prep_inputs(hidden_states, freqs_cis, attention_mask, wq, wk, wv, wo):
    hid = np.asarray(hidden_states, np.float32).reshape(S, D)
    hidT = np.ascontiguousarray(hid.T).astype(bf16)

    # within-head channel permutation: evens then odds (rope pairs 32 apart)
    perm1 = np.concatenate([np.arange(0, HD, 2), np.arange(1, HD, 2)])
    perm = np.concatenate([perm1, perm1 + HD])  # for the 2 heads of a core

    fc = np.asarray(freqs_cis, np.float32)
    cosT = np.ascontiguousarray(fc[:, :, 0].T)  # [32, S]
    sinT = np.ascontiguousarray(fc[:, :, 1].T)
    cosf = np.concatenate([cosT, cosT, cosT, cosT], 0).astype(bf16)
    sinf = np.concatenate([-sinT, sinT, -sinT, sinT], 0).astype(bf16)

    mask_add = (1.0 - np.asarray(attention_mask, np.float32).reshape(S)) * -10000.0
    maskadd = np.ascontiguousarray(mask_add.reshape(NKC, 128).T).astype(np.float32)

    def wlayout(w):  # [1024, 128] -> [128 partitions, chunk-major 1024]
        w = np.ascontiguousarray(w)
        return np.ascontiguousarray(
            w.reshape(NDC, 128, 128).transpose(1, 0, 2).reshape(128, D)
        ).astype(bf16)

    in_maps = []
    for core in range(8):
        cols = slice(core * 128, (core + 1) * 128)
        in_maps.append(
            {
                "hidT": hidT,
                "wq": wlayout(np.asarray(wq, np.float32)[:, cols][:, perm]),
                "wk": wlayout(np.asarray(wk, np.float32)[:, cols][:, perm]),
                "wv": wlayout(np.asarray(wv, np.float32)[:, cols]),
                "wo": np.ascontiguousarray(np.asarray(wo, np.float32)[cols, :]).astype(bf16),
                "cosf": cosf,
                "sinf": sinf,
                "maskadd": maskadd,
                "ones64": np.ones((1, 64), np.float32),
            }
        )
    return in_maps


def run_sharded(in_maps, **kwargs):
    nc = _cached_program()
    return run_bass_kernel_spmd(nc, in_maps, list(range(8)), **kwargs)


def kernel(hidden_states, freqs_cis, attention_mask, wq, wk, wv, wo):
    in_maps = _prep_inputs(
        hidden_states, freqs_cis, attention_mask, wq, wk, wv, wo
    )
    res = run_sharded(in_maps).results
    out = np.zeros((S, D), np.float32)
    for r in res:
        out += np.asarray(r["outp"], np.float32)
    return out.reshape(1, S, D)


if __name__ == "__main__":
    import reference

    inputs = reference.setup_inputs()
    inputs = {k: np.asarray(v) for k, v in inputs.items()}
    expected = np.asarray(reference.reference(**inputs))
    actual = kernel(**inputs)
    err = np.abs(actual - expected).max() / np.abs(expected).max()
    print("Relative error:", err)


# revision 16
# speedup vs baseline: 1.7515x; 1.0264x over previous
"""Trainium2 Bass kernel for 16-head MHA with RoPE (B=1, S=4096, D=1024).

Sharding: tensor-parallel over heads — 2 heads per core on 8 cores.
Per-core pipeline (all matmuls bf16, fp32 PSUM accumulation):
  1. Load hidden transposed [d, s] (host-prepared bf16) + weight slices.
  2. Projections in order Q, V, K; per projection the weight chunks are
     stationary and hidT streams; PSUM copied straight to bf16 SBUF.
  3. RoPE on q/k in bf16 via the partition-swap trick (channels permuted
     host-side to [evens | odds] per head). K is roped in 512-col segments
     so attention can start as soon as the first segments land.
  4. V transposed (xbar DMA) to v_nat [s, c], scaled by f[k]=exp(mask_add),
     then restrided into 66-col blocks per (chunk, head) with f appended as
     a 65th stationary column — the ctx matmul then accumulates the softmax
     denominator into PSUM partition 64 for free (no separate den matmuls).
  5. Attention per q-tile of 512, transposed scores S_T[k, q]: exp on
     ScalarE (scale=1/8 folded), ctx+den accumulate over 32 k-chunks into
     two [65, 512] PSUM banks (one per head). Scores double-buffered with
     one-group lookahead so the PE never stalls on the exp.
  6. Denominator reciprocals via reciprocal_approx_fast straight from PSUM,
     broadcast across partitions with a rank-1 f32r PE matmul into a
     borrowed score-ring bank; fused normalize+cast to bf16.
  7. Out-projection of q-tile t interleaved into q-tile t+1 through the
     same PSUM ring; partials written to DRAM as fp16.
Host sums the 8 fp16 partials in fp32.
"""

import functools

import numpy as np
import ml_dtypes

import concourse.bass as bass
import concourse.tile as tile
import concourse.mybir as mybir
from concourse.bass_utils import run_bass_kernel_spmd

BF16 = mybir.dt.bfloat16
F16 = mybir.dt.float16
F32 = mybir.dt.float32
F32R = mybir.dt.float32r
bf16 = ml_dtypes.bfloat16

S = 4096      # sequence length
D = 1024      # model dim
HD = 64       # head dim
C = 128       # channels per core (2 heads)
NDC = 8       # contraction chunks of 128 over D
NKC = 32      # key chunks of 128 over S
NQT = 8       # query tiles of 512
QT = 512
GRP = 3       # score (chunk, head) slots per exp group / PSUM bank triple
VB = 66       # v4 block stride per (chunk, head): 64 V cols + f col + pad


_NO_SPLIT = (
    mybir.InstEventSemaphore,
    mybir.InstUnconditionalBranch,
)


def _split_multi_waits(nc: bass.Bass) -> None:
    """Hoist extra sem waits onto standalone EventSemaphore carriers.

    This walrus build only supports one sync-wait command per engine
    instruction ("Too many sync wait commands" in setupSyncWait), so any
    instruction Tile scheduled with >1 wait gets all but its last wait moved
    to dedicated InstEventSemaphore instructions placed immediately before it
    in the same engine stream (sequencer blocks on them in program order —
    semantically identical).
    """
    n = 0
    for fn in nc.m.functions:
        for blk in fn.blocks:
            out = []
            for inst in blk.instructions:
                si = inst.sync_info
                if (
                    si is not None
                    and si.on_wait
                    and len(si.on_wait) > 1
                    and not isinstance(inst, _NO_SPLIT)
                    and inst.engine != mybir.EngineType.Unassigned
                ):
                    waits = list(si.on_wait)
                    for w in waits[:-1]:
                        ev = mybir.InstEventSemaphore(name=f"ant_waitsplit_{n}")
                        n += 1
                        ev.engine = inst.engine
                        ev.sync_info = mybir.SyncInfo(on_wait=[w], on_update=[])
                        nc.register_instruction(ev)
                        out.append(ev)
                    si.on_wait = [waits[-1]]
                    inst.sync_info = si
                out.append(inst)
            blk.instructions[:] = out


def build_program() -> bass.Bass:
    nc = bass.Bass()
    hidT_d = nc.declare_dram_parameter("hidT", [D, S], BF16, isOutput=False)
    wq_d = nc.declare_dram_parameter("wq", [128, D], BF16, isOutput=False)
    wk_d = nc.declare_dram_parameter("wk", [128, D], BF16, isOutput=False)
    wv_d = nc.declare_dram_parameter("wv", [128, D], BF16, isOutput=False)
    wo_d = nc.declare_dram_parameter("wo", [128, D], BF16, isOutput=False)
    cos_d = nc.declare_dram_parameter("cosf", [128, S], BF16, isOutput=False)
    sin_d = nc.declare_dram_parameter("sinf", [128, S], BF16, isOutput=False)
    mask_d = nc.declare_dram_parameter("maskadd", [128, NKC], F32, isOutput=False)
    ones_d = nc.declare_dram_parameter("ones64", [1, 64], F32, isOutput=False)
    out_d = nc.declare_dram_parameter("outp", [S, D], F16, isOutput=True)

    Exp = mybir.ActivationFunctionType.Exp
    mult = mybir.AluOpType.mult
    add = mybir.AluOpType.add

    with tile.TileContext(nc) as tc:
        with (
            tc.tile_pool(name="const", bufs=1) as const,
            tc.tile_pool(name="ppool", bufs=3) as ppool,
        ):
            # ---- persistent SBUF tiles -------------------------------------
            wq_sb = const.tile([128, D], BF16, tag="wq")
            wk_sb = const.tile([128, D], BF16, tag="wk")
            wv_sb = const.tile([128, D], BF16, tag="wv")
            wo_sb = const.tile([128, D], BF16, tag="wo")
            mask_sb = const.tile([128, NKC], F32, tag="mask")
            f_f32 = const.tile([128, NKC], F32, tag="ff32")
            ones_sb = const.tile([1, 64], F32, tag="ones")
            cos_sb = const.tile([128, S], BF16, tag="cosf")
            sin_sb = const.tile([128, S], BF16, tag="sinf")
            qT_bf = const.tile([128, S], BF16, tag="qTbf")
            kT_bf = const.tile([128, S], BF16, tag="kTbf")
            qsw = const.tile([128, S], BF16, tag="qsw")
            v4 = const.tile([128, NKC * 2 * VB], BF16, tag="v4")
            ctxn = const.tile([128, S], BF16, tag="ctxn")

            nc.sync.dma_start(out=wq_sb[:], in_=wq_d[:])
            nc.sync.dma_start(out=wv_sb[:], in_=wv_d[:])
            nc.sync.dma_start(out=wk_sb[:], in_=wk_d[:])
            nc.sync.dma_start(out=wo_sb[:], in_=wo_d[:])
            nc.sync.dma_start(out=mask_sb[:], in_=mask_d[:])
            nc.sync.dma_start(out=ones_sb[:], in_=ones_d[:])
            nc.sync.dma_start(out=cos_sb[:], in_=cos_d[:])
            nc.sync.dma_start(out=sin_sb[:], in_=sin_d[:])
            # f[k] = exp(mask_add[k]) — also warms the ACT exp table early
            nc.scalar.activation(f_f32[:], mask_sb[:], Exp)

            # ---- phase 1: load hidT + projections + rope -------------------
            with (
                tc.tile_pool(name="hid", bufs=1) as hid,
                tc.tile_pool(name="projps", bufs=1, space="PSUM") as projps,
            ):
                hidT_sb = hid.tile([128, NDC * S], BF16, tag="hidT")
                for dc in range(NDC):
                    nc.sync.dma_start(
                        out=hidT_sb[:, dc * S : (dc + 1) * S],
                        in_=hidT_d[dc * 128 : (dc + 1) * 128, :],
                    )
                vT_bf = hid.tile([128, S], BF16, tag="vTbf")
                v_nat = hid.tile([128, S], BF16, tag="vnat")

                ps = [
                    projps.tile([128, QT], F32, name=f"pj{st}", tag=f"pj{st}")
                    for st in range(8)
                ]

                def project(w_sb, dst, st_outer=False, per_st=None, cp=None):
                    # dc-outer consumes hidT chunks as they stream in (for the
                    # first, DMA-paced projection); st-outer finishes one
                    # output tile at a time so its PSUM copy (and optional
                    # per-tile continuation like rope) can start immediately.
                    # cp: engine hook doing the PSUM->SBUF copy (ScalarE for
                    # q/v frees the DVE for rope; DVE for k keeps the
                    # copy+rope segment chain on one engine).
                    if cp is None:
                        cp = nc.scalar.copy
                    if st_outer:
                        for st in range(8):
                            for dc in range(NDC):
                                nc.tensor.matmul(
                                    ps[st][:],
                                    lhsT=w_sb[:, dc * 128 : (dc + 1) * 128],
                                    rhs=hidT_sb[:, dc * S + st * QT : dc * S + (st + 1) * QT],
                                    start=(dc == 0),
                                    stop=(dc == NDC - 1),
                                )
                            cp(dst[:, st * QT : (st + 1) * QT], ps[st][:])
                            if per_st is not None:
                                per_st(st)
                    else:
                        for dc in range(NDC):
                            for st in range(8):
                                nc.tensor.matmul(
                                    ps[st][:],
                                    lhsT=w_sb[:, dc * 128 : (dc + 1) * 128],
                                    rhs=hidT_sb[:, dc * S + st * QT : dc * S + (st + 1) * QT],
                                    start=(dc == 0),
                                    stop=(dc == NDC - 1),
                                )
                        for st in range(8):
                            cp(dst[:, st * QT : (st + 1) * QT], ps[st][:])

                def rope(x_bf, s0, s1):
                    # channel rows per head h: [h*64, h*64+32) = evens ("a"),
                    # [h*64+32, h*64+64) = odds ("b");
                    # out = x * cos_full + swap(x) * sin_signed
                    sc = slice(s0, s1)
                    for h in range(2):
                        a = slice(h * 64, h * 64 + 32)
                        b = slice(h * 64 + 32, h * 64 + 64)
                        nc.vector.tensor_copy(qsw[a, sc], x_bf[b, sc])
                        nc.vector.tensor_copy(qsw[b, sc], x_bf[a, sc])
                    nc.vector.tensor_tensor(x_bf[:, sc], x_bf[:, sc], cos_sb[:, sc], op=mult)
                    nc.vector.tensor_tensor(qsw[:, sc], qsw[:, sc], sin_sb[:, sc], op=mult)
                    nc.vector.tensor_tensor(x_bf[:, sc], x_bf[:, sc], qsw[:, sc], op=add)

                # Q projection (dc-outer: paced by the hidT DMA stream) +
                # full-width rope
                project(wq_sb, qT_bf)
                rope(qT_bf, 0, S)

                # V projection (st-outer: copies start early) + transpose +
                # f-scale + restride into v4 — the post-transpose V pipeline
                # runs on the otherwise-idle ScalarE
                project(wv_sb, vT_bf, st_outer=True)
                nc.sync.dma_start_transpose(
                    out=v_nat[:].rearrange("p (kc c) -> p kc c", kc=NKC),
                    in_=vT_bf[:],
                )
                Copy = mybir.ActivationFunctionType.Copy
                for kc in range(NKC):
                    nc.scalar.activation(
                        v_nat[:, kc * 128 : (kc + 1) * 128],
                        v_nat[:, kc * 128 : (kc + 1) * 128],
                        Copy,
                        scale=f_f32[:, kc : kc + 1],
                    )
                v4r = v4[:].rearrange("p (kc h c) -> p kc h c", kc=NKC, h=2)
                vnr = v_nat[:].rearrange("p (kc h c) -> p kc h c", kc=NKC, h=2)
                nc.scalar.copy(v4r[:, :, :, 0:64], vnr[:, :, :, :])
                for h in range(2):
                    nc.scalar.copy(
                        v4r[:, :, h : h + 1, 64:65],
                        f_f32[:].unsqueeze(-1).unsqueeze(-1),
                    )

                # K projection st-outer with copy+rope fused per 512-col tile
                # on the DVE — attention scores can start on chunk 0 while
                # later K tiles are still in the PE
                project(
                    wk_sb,
                    kT_bf,
                    st_outer=True,
                    per_st=lambda st: rope(kT_bf, st * QT, (st + 1) * QT),
                    cp=nc.vector.tensor_copy,
                )

            # ---- phase 2: attention + fused out-projection -----------------
            slots = [(c, h) for c in range(NKC) for h in range(2)]
            groups = [slots[i : i + GRP] for i in range(0, len(slots), GRP)]
            NG = len(groups)  # 22

            with (
                tc.tile_pool(name="sgps", bufs=2, space="PSUM") as sgps,
                tc.tile_pool(name="ctxps", bufs=1, space="PSUM") as ctxps,
                tc.tile_pool(name="rpool", bufs=2) as rpool,
                tc.tile_pool(name="opool", bufs=3) as opool,
            ):
                ctxA = ctxps.tile([65, QT], F32, tag="ctxA")
                ctxB = ctxps.tile([65, QT], F32, tag="ctxB")
                ctx_banks = (ctxA, ctxB)

                # pre-fill both dsb ring buffers so the single batched
                # reciprocal's untouched rows (1..31) stay finite
                for _ in range(2):
                    dpre = rpool.tile([33, QT], F32, tag="dsb", name="dpre")
                    nc.vector.memset(dpre[:], 1.0)

                def emit_scores(qt, g):
                    qc = slice(qt * QT, (qt + 1) * QT)
                    grp = groups[g]
                    nb = len(grp)
                    sg = sgps.tile([128, GRP * QT], F32, tag="sg")
                    Pt = ppool.tile([128, GRP * QT], BF16, tag="pt")
                    for i, (c, h) in enumerate(grp):
                        hr = slice(h * 64, (h + 1) * 64)
                        nc.tensor.matmul(
                            sg[:, i * QT : (i + 1) * QT],
                            lhsT=kT_bf[hr, c * 128 : (c + 1) * 128],
                            rhs=qT_bf[hr, qc],
                            start=True,
                            stop=True,
                        )
                    nc.scalar.activation(
                        Pt[:, : nb * QT], sg[:, : nb * QT], Exp, scale=0.125
                    )
                    return Pt

                def emit_ctx(g, Pt):
                    grp = groups[g]
                    for i, (c, h) in enumerate(grp):
                        vcol = (c * 2 + h) * VB
                        nc.tensor.matmul(
                            ctx_banks[h][:, :],
                            lhsT=v4[:, vcol : vcol + 65],
                            rhs=Pt[:, i * QT : (i + 1) * QT],
                            start=(c == 0),
                            stop=(c == NKC - 1),
                        )

                def emit_recips(qt):
                    # Evacuate the ctx banks to SBUF right after the final ctx
                    # accumulation (frees the banks for the next qtile fast),
                    # then the reciprocal of the two denominator rows runs off
                    # the PSUM-WAR critical path.
                    dsb = rpool.tile([33, QT], F32, tag="dsb")
                    nc.vector.tensor_copy(dsb[0:1, :], ctxA[64:65, :])
                    nc.vector.tensor_copy(dsb[32:33, :], ctxB[64:65, :])
                    ctxc = rpool.tile([128, QT], F32, tag="ctxc")
                    nc.vector.tensor_copy(ctxc[0:64, :], ctxA[0:64, :])
                    nc.vector.tensor_copy(ctxc[64:128, :], ctxB[0:64, :])
                    rcp = rpool.tile([33, QT], F32, tag="rcp")
                    # one call covers both denominator rows (0 and 32); the
                    # uninitialized rows in between are never read
                    nc.vector.reciprocal(rcp[:, :], dsb[:, :])
                    rsb = rpool.tile([1, 2 * QT], F32, tag="rsb")
                    nc.vector.tensor_copy(rsb[0:1, 0:QT], rcp[0:1, :])
                    nc.vector.tensor_copy(rsb[0:1, QT : 2 * QT], rcp[32:33, :])
                    return rsb, ctxc

                def emit_normalize(qt, rsb, ctxc):
                    # broadcast recips across partitions via rank-1 PE matmuls
                    # into a borrowed score-ring bank, then fused
                    # normalize+cast into ctxn (all SBUF inputs — no ctx-bank
                    # dependence left).
                    qc = slice(qt * QT, (qt + 1) * QT)
                    bt = sgps.tile([128, GRP * QT], F32, tag="sg")
                    nc.tensor.matmul(
                        bt[0:64, 0:QT],
                        lhsT=ones_sb[:],
                        rhs=rsb[0:1, 0:QT],
                        start=True,
                        stop=True,
                    )
                    nc.tensor.matmul(
                        bt[64:128, 0:QT],
                        lhsT=ones_sb[:],
                        rhs=rsb[0:1, QT : 2 * QT],
                        start=True,
                        stop=True,
                        tile_position=(0, 64),
                    )
                    recb = rpool.tile([128, QT], F32, tag="recb")
                    nc.vector.tensor_copy(recb[:], bt[:, 0:QT])
                    nc.vector.tensor_tensor(
                        ctxn[0:64, qc], ctxc[0:64, :], recb[0:64, :], op=mult
                    )
                    nc.vector.tensor_tensor(
                        ctxn[64:128, qc], ctxc[64:128, :], recb[64:128, :], op=mult
                    )

                def emit_outproj(qt, j):
                    # out rows [qt*512 + j*128, +128) = ctxn_chunk^T @ wo
                    ot = sgps.tile([128, GRP * QT], F32, tag="sg")
                    col = qt * QT + j * 128
                    for half in range(2):
                        nc.tensor.matmul(
                            ot[:, half * QT : (half + 1) * QT],
                            lhsT=ctxn[:, col : col + 128],
                            rhs=wo_sb[:, half * QT : (half + 1) * QT],
                            start=True,
                            stop=True,
                        )
                    osb = opool.tile([128, D], F16, tag="osb")
                    nc.vector.tensor_copy(osb[:], ot[:, 0:D])
                    nc.sync.dma_start(
                        out=out_d[col : col + 128, :], in_=osb[:]
                    )

                norm_prev = None
                for qt in range(NQT):
                    pts = {}
                    for g in range(NG):
                        pts[g] = emit_scores(qt, g)
                        if g == 4 and qt > 0:
                            emit_normalize(qt - 1, *norm_prev)
                        if qt > 0 and g in (7, 12, 16, 21):
                            emit_outproj(qt - 1, {7: 0, 12: 1, 16: 2, 21: 3}[g])
                        if g >= 1:
                            emit_ctx(g - 1, pts.pop(g - 1))
                    emit_ctx(NG - 1, pts.pop(NG - 1))
                    norm_prev = emit_recips(qt)
                emit_normalize(NQT - 1, *norm_prev)
                for j in range(4):
                    emit_outproj(NQT - 1, j)

    _split_multi_waits(nc)
    return nc


@functools.cache
def _cached_program() -> bass.Bass:
    return build_program()


def _prep_inputs(hidden_states, freqs_cis, attention_mask, wq, wk, wv, wo):
    hid = np.asarray(hidden_states, np.float32).reshape(S, D)
    hidT = np.ascontiguousarray(hid.T).astype(bf16)

    # within-head channel permutation: evens then odds (rope pairs 32 apart)
    perm1 = np.concatenate([np.arange(0, HD, 2), np.arange(1, HD, 2)])
    perm = np.concatenate([perm1, perm1 + HD])  # for the 2 heads of a core

    fc = np.asarray(freqs_cis, np.float32)
    cosT = np.ascontiguousarray(fc[:, :, 0].T)  # [32, S]
    sinT = np.ascontiguousarray(fc[:, :, 1].T)
    cosf = np.concatenate([cosT, cosT, cosT, cosT], 0).astype(bf16)
    sinf = np.concatenate([-sinT, sinT, -sinT, sinT], 0).astype(bf16)

    mask_add = (1.0 - np.asarray(attention_mask, np.float32).reshape(S)) * -10000.0
    maskadd = np.ascontiguousarray(mask_add.reshape(NKC, 128).T).astype(np.float32)

    def wlayout(w):  # [1024, 128] -> [128 partitions, chunk-major 1024]
        w = np.ascontiguousarray(w)
        return np.ascontiguousarray(
            w.reshape(NDC, 128, 128).transpose(1, 0, 2).reshape(128, D)
        ).astype(bf16)

    in_maps = []
    for core in range(8):
        cols = slice(core * 128, (core + 1) * 128)
        in_maps.append(
            {
                "hidT": hidT,
                "wq": wlayout(np.asarray(wq, np.float32)[:, cols][:, perm]),
                "wk": wlayout(np.asarray(wk, np.float32)[:, cols][:, perm]),
                "wv": wlayout(np.asarray(wv, np.float32)[:, cols]),
                "wo": np.ascontiguousarray(np.asarray(wo, np.float32)[cols, :]).astype(bf16),
                "cosf": cosf,
                "sinf": sinf,
                "maskadd": maskadd,
                "ones64": np.ones((1, 64), np.float32),
            }
        )
    return in_maps


def run_sharded(in_maps, **kwargs):
    nc = _cached_program()
    return run_bass_kernel_spmd(nc, in_maps, list(range(8)), **kwargs)


def kernel(hidden_states, freqs_cis, attention_mask, wq, wk, wv, wo):
    in_maps = _prep_inputs(
        hidden_states, freqs_cis, attention_mask, wq, wk, wv, wo
    )
    res = run_sharded(in_maps).results
    out = np.zeros((S, D), np.float32)
    for r in res:
        out += np.asarray(r["outp"], np.float32)
    return out.reshape(1, S, D)


if __name__ == "__main__":
    import reference

    inputs = reference.setup_inputs()
    inputs = {k: np.asarray(v) for k, v in inputs.items()}
    expected = np.asarray(reference.reference(**inputs))
    actual = kernel(**inputs)
    err = np.abs(actual - expected).max() / np.abs(expected).max()
    print("Relative error:", err)


# revision 18
# speedup vs baseline: 1.7818x; 1.0173x over previous
"""Trainium2 Bass kernel for 16-head MHA with RoPE (B=1, S=4096, D=1024).

Sharding: tensor-parallel over heads — 2 heads per core on 8 cores.
Per-core pipeline (all matmuls bf16, fp32 PSUM accumulation):
  1. Load hidden transposed [d, s] (host-prepared bf16) + weight slices.
  2. Projections in order Q, V, K; per projection the weight chunks are
     stationary and hidT streams; PSUM copied straight to bf16 SBUF.
  3. RoPE on q/k in bf16 via the partition-swap trick (channels permuted
     host-side to [evens | odds] per head). K is roped in 512-col segments
     so attention can start as soon as the first segments land.
  4. V transposed (xbar DMA) to v_nat [s, c], scaled by f[k]=exp(mask_add),
     then restrided into 66-col blocks per (chunk, head) with f appended as
     a 65th stationary column — the ctx matmul then accumulates the softmax
     denominator into PSUM partition 64 for free (no separate den matmuls).
  5. Attention per q-tile of 512, transposed scores S_T[k, q]: exp on
     ScalarE (scale=1/8 folded), ctx+den accumulate over 32 k-chunks into
     two [65, 512] PSUM banks (one per head). Scores double-buffered with
     one-group lookahead so the PE never stalls on the exp.
  6. Denominator reciprocals via reciprocal_approx_fast straight from PSUM,
     broadcast across partitions with a rank-1 f32r PE matmul into a
     borrowed score-ring bank; fused normalize+cast to bf16.
  7. Out-projection of q-tile t interleaved into q-tile t+1 through the
     same PSUM ring; partials written to DRAM as fp16.
Host sums the 8 fp16 partials in fp32.
"""

import functools

import numpy as np
import ml_dtypes

import concourse.bass as bass
import concourse.tile as tile
import concourse.mybir as mybir
from concourse.bass_utils import run_bass_kernel_spmd

BF16 = mybir.dt.bfloat16
F16 = mybir.dt.float16
F32 = mybir.dt.float32
F32R = mybir.dt.float32r
bf16 = ml_dtypes.bfloat16

S = 4096      # sequence length
D = 1024      # model dim
HD = 64       # head dim
C = 128       # channels per core (2 heads)
NDC = 8       # contraction chunks of 128 over D
NKC = 32      # key chunks of 128 over S
NQT = 8       # query tiles of 512
QT = 512
GRP = 3       # score (chunk, head) slots per exp group / PSUM bank triple
VB = 66       # v4 block stride per (chunk, head): 64 V cols + f col + pad


_NO_SPLIT = (
    mybir.InstEventSemaphore,
    mybir.InstUnconditionalBranch,
)


def _split_multi_waits(nc: bass.Bass) -> None:
    """Hoist extra sem waits onto standalone EventSemaphore carriers.

    This walrus build only supports one sync-wait command per engine
    instruction ("Too many sync wait commands" in setupSyncWait), so any
    instruction Tile scheduled with >1 wait gets all but its last wait moved
    to dedicated InstEventSemaphore instructions placed immediately before it
    in the same engine stream (sequencer blocks on them in program order —
    semantically identical).
    """
    n = 0
    for fn in nc.m.functions:
        for blk in fn.blocks:
            out = []
            for inst in blk.instructions:
                si = inst.sync_info
                if (
                    si is not None
                    and si.on_wait
                    and len(si.on_wait) > 1
                    and not isinstance(inst, _NO_SPLIT)
                    and inst.engine != mybir.EngineType.Unassigned
                ):
                    waits = list(si.on_wait)
                    for w in waits[:-1]:
                        ev = mybir.InstEventSemaphore(name=f"ant_waitsplit_{n}")
                        n += 1
                        ev.engine = inst.engine
                        ev.sync_info = mybir.SyncInfo(on_wait=[w], on_update=[])
                        nc.register_instruction(ev)
                        out.append(ev)
                    si.on_wait = [waits[-1]]
                    inst.sync_info = si
                out.append(inst)
            blk.instructions[:] = out


def build_program() -> bass.Bass:
    nc = bass.Bass()
    hidT_d = nc.declare_dram_parameter("hidT", [D, S], BF16, isOutput=False)
    wq_d = nc.declare_dram_parameter("wq", [128, D], BF16, isOutput=False)
    wk_d = nc.declare_dram_parameter("wk", [128, D], BF16, isOutput=False)
    wv_d = nc.declare_dram_parameter("wv", [128, D], BF16, isOutput=False)
    wo_d = nc.declare_dram_parameter("wo", [128, D], BF16, isOutput=False)
    cos_d = nc.declare_dram_parameter("cosf", [128, S], BF16, isOutput=False)
    sin_d = nc.declare_dram_parameter("sinf", [128, S], BF16, isOutput=False)
    mask_d = nc.declare_dram_parameter("maskadd", [128, NKC], F32, isOutput=False)
    ones_d = nc.declare_dram_parameter("ones64", [1, 64], F32, isOutput=False)
    out_d = nc.declare_dram_parameter("outp", [S, D], F16, isOutput=True)

    Exp = mybir.ActivationFunctionType.Exp
    mult = mybir.AluOpType.mult
    add = mybir.AluOpType.add

    with tile.TileContext(nc) as tc:
        with (
            tc.tile_pool(name="const", bufs=1) as const,
            tc.tile_pool(name="ppool", bufs=3) as ppool,
        ):
            # ---- persistent SBUF tiles -------------------------------------
            wq_sb = const.tile([128, D], BF16, tag="wq")
            wk_sb = const.tile([128, D], BF16, tag="wk")
            wv_sb = const.tile([128, D], BF16, tag="wv")
            wo_sb = const.tile([128, D], BF16, tag="wo")
            mask_sb = const.tile([128, NKC], F32, tag="mask")
            f_f32 = const.tile([128, NKC], F32, tag="ff32")
            ones_sb = const.tile([1, 64], F32, tag="ones")
            cos_sb = const.tile([128, S], BF16, tag="cosf")
            sin_sb = const.tile([128, S], BF16, tag="sinf")
            qT_bf = const.tile([128, S], BF16, tag="qTbf")
            kT_bf = const.tile([128, S], BF16, tag="kTbf")
            qsw = const.tile([128, S], BF16, tag="qsw")
            v4 = const.tile([128, NKC * 2 * VB], BF16, tag="v4")
            ctxn = const.tile([128, S], BF16, tag="ctxn")

            # sync queue carries wq + the hidT stream (critical path for the
            # first projection); everything else rides the scalar hwdge queue
            nc.sync.dma_start(out=wq_sb[:], in_=wq_d[:])
            nc.scalar.dma_start(out=mask_sb[:], in_=mask_d[:])
            nc.scalar.dma_start(out=ones_sb[:], in_=ones_d[:])
            nc.scalar.dma_start(out=wv_sb[:], in_=wv_d[:])
            nc.scalar.dma_start(out=wk_sb[:], in_=wk_d[:])
            nc.scalar.dma_start(out=wo_sb[:], in_=wo_d[:])
            nc.scalar.dma_start(out=cos_sb[:], in_=cos_d[:])
            nc.scalar.dma_start(out=sin_sb[:], in_=sin_d[:])
            # f[k] = exp(mask_add[k]) — also warms the ACT exp table early
            nc.scalar.activation(f_f32[:], mask_sb[:], Exp)

            # ---- phase 1: load hidT + projections + rope -------------------
            with (
                tc.tile_pool(name="hid", bufs=1) as hid,
                tc.tile_pool(name="projps", bufs=1, space="PSUM") as projps,
            ):
                hidT_sb = hid.tile([128, NDC * S], BF16, tag="hidT")
                for dc in range(NDC):
                    nc.sync.dma_start(
                        out=hidT_sb[:, dc * S : (dc + 1) * S],
                        in_=hidT_d[dc * 128 : (dc + 1) * 128, :],
                    )
                vT_bf = hid.tile([128, S], BF16, tag="vTbf")
                v_nat = hid.tile([128, S], BF16, tag="vnat")

                ps = [
                    projps.tile([128, QT], F32, name=f"pj{st}", tag=f"pj{st}")
                    for st in range(8)
                ]

                def project(w_sb, dst, st_outer=False, per_st=None, cp=None):
                    # dc-outer consumes hidT chunks as they stream in (for the
                    # first, DMA-paced projection); st-outer finishes one
                    # output tile at a time so its PSUM copy (and optional
                    # per-tile continuation like rope) can start immediately.
                    # cp: engine hook doing the PSUM->SBUF copy (ScalarE for
                    # q/v frees the DVE for rope; DVE for k keeps the
                    # copy+rope segment chain on one engine).
                    if cp is None:
                        cp = nc.scalar.copy
                    if st_outer:
                        for st in range(8):
                            for dc in range(NDC):
                                nc.tensor.matmul(
                                    ps[st][:],
                                    lhsT=w_sb[:, dc * 128 : (dc + 1) * 128],
                                    rhs=hidT_sb[:, dc * S + st * QT : dc * S + (st + 1) * QT],
                                    start=(dc == 0),
                                    stop=(dc == NDC - 1),
                                )
                            cp(dst[:, st * QT : (st + 1) * QT], ps[st][:])
                            if per_st is not None:
                                per_st(st)
                    else:
                        for dc in range(NDC):
                            for st in range(8):
                                nc.tensor.matmul(
                                    ps[st][:],
                                    lhsT=w_sb[:, dc * 128 : (dc + 1) * 128],
                                    rhs=hidT_sb[:, dc * S + st * QT : dc * S + (st + 1) * QT],
                                    start=(dc == 0),
                                    stop=(dc == NDC - 1),
                                )
                        for st in range(8):
                            cp(dst[:, st * QT : (st + 1) * QT], ps[st][:])

                def rope(x_bf, s0, s1):
                    # channel rows per head h: [h*64, h*64+32) = evens ("a"),
                    # [h*64+32, h*64+64) = odds ("b");
                    # out = x * cos_full + swap(x) * sin_signed
                    sc = slice(s0, s1)
                    for h in range(2):
                        a = slice(h * 64, h * 64 + 32)
                        b = slice(h * 64 + 32, h * 64 + 64)
                        nc.vector.tensor_copy(qsw[a, sc], x_bf[b, sc])
                        nc.vector.tensor_copy(qsw[b, sc], x_bf[a, sc])
                    nc.vector.tensor_tensor(x_bf[:, sc], x_bf[:, sc], cos_sb[:, sc], op=mult)
                    nc.vector.tensor_tensor(qsw[:, sc], qsw[:, sc], sin_sb[:, sc], op=mult)
                    nc.vector.tensor_tensor(x_bf[:, sc], x_bf[:, sc], qsw[:, sc], op=add)

                # Q projection (dc-outer: paced by the hidT DMA stream) +
                # full-width rope
                project(wq_sb, qT_bf)
                rope(qT_bf, 0, S)

                # V projection (st-outer: copies start early) + transpose,
                # then a single fused scale+restride pass into v4 (f folded
                # into the copy): the first 8 chunks on ScalarE (just in time
                # for the first ctx matmuls), the rest on the DVE after the
                # K chain.
                project(wv_sb, vT_bf, st_outer=True)
                nc.sync.dma_start_transpose(
                    out=v_nat[:].rearrange("p (kc c) -> p kc c", kc=NKC),
                    in_=vT_bf[:],
                )
                Copy = mybir.ActivationFunctionType.Copy
                v4r = v4[:].rearrange("p (kc h c) -> p kc h c", kc=NKC, h=2)
                vnr = v_nat[:].rearrange("p (kc h c) -> p kc h c", kc=NKC, h=2)
                for kc in range(8):
                    nc.scalar.activation(
                        v4r[:, kc : kc + 1, :, 0:64],
                        vnr[:, kc : kc + 1, :, :],
                        Copy,
                        scale=f_f32[:, kc : kc + 1],
                    )
                for h in range(2):
                    nc.scalar.copy(
                        v4r[:, :, h : h + 1, 64:65],
                        f_f32[:].unsqueeze(-1).unsqueeze(-1),
                    )

                # K projection st-outer with copy+rope fused per 512-col tile
                # on the DVE — attention scores can start on chunk 0 while
                # later K tiles are still in the PE
                project(
                    wk_sb,
                    kT_bf,
                    st_outer=True,
                    per_st=lambda st: rope(kT_bf, st * QT, (st + 1) * QT),
                    cp=nc.vector.tensor_copy,
                )

                # remaining v4 chunks on the DVE (ctx consumes ~1 chunk per
                # microsecond, so these stay comfortably ahead)
                for kc in range(8, NKC):
                    nc.vector.tensor_scalar(
                        v4r[:, kc : kc + 1, :, 0:64],
                        vnr[:, kc : kc + 1, :, :],
                        f_f32[:, kc : kc + 1],
                        None,
                        op0=mult,
                    )

            # ---- phase 2: attention + fused out-projection -----------------
            slots = [(c, h) for c in range(NKC) for h in range(2)]
            groups = [slots[i : i + GRP] for i in range(0, len(slots), GRP)]
            NG = len(groups)  # 22

            with (
                tc.tile_pool(name="sgps", bufs=2, space="PSUM") as sgps,
                tc.tile_pool(name="ctxps", bufs=1, space="PSUM") as ctxps,
                tc.tile_pool(name="rpool", bufs=2) as rpool,
                tc.tile_pool(name="opool", bufs=3) as opool,
            ):
                ctxA = ctxps.tile([65, QT], F32, tag="ctxA")
                ctxB = ctxps.tile([65, QT], F32, tag="ctxB")
                ctx_banks = (ctxA, ctxB)

                # pre-fill both dsb ring buffers so the single batched
                # reciprocal's untouched rows (1..31) stay finite
                for _ in range(2):
                    dpre = rpool.tile([33, QT], F32, tag="dsb", name="dpre")
                    nc.vector.memset(dpre[:], 1.0)

                def emit_scores(qt, g):
                    qc = slice(qt * QT, (qt + 1) * QT)
                    grp = groups[g]
                    nb = len(grp)
                    sg = sgps.tile([128, GRP * QT], F32, tag="sg")
                    Pt = ppool.tile([128, GRP * QT], BF16, tag="pt")
                    for i, (c, h) in enumerate(grp):
                        hr = slice(h * 64, (h + 1) * 64)
                        nc.tensor.matmul(
                            sg[:, i * QT : (i + 1) * QT],
                            lhsT=kT_bf[hr, c * 128 : (c + 1) * 128],
                            rhs=qT_bf[hr, qc],
                            start=True,
                            stop=True,
                        )
                    nc.scalar.activation(
                        Pt[:, : nb * QT], sg[:, : nb * QT], Exp, scale=0.125
                    )
                    return Pt

                def emit_ctx(g, Pt):
                    grp = groups[g]
                    for i, (c, h) in enumerate(grp):
                        vcol = (c * 2 + h) * VB
                        nc.tensor.matmul(
                            ctx_banks[h][:, :],
                            lhsT=v4[:, vcol : vcol + 65],
                            rhs=Pt[:, i * QT : (i + 1) * QT],
                            start=(c == 0),
                            stop=(c == NKC - 1),
                        )

                def emit_recips(qt):
                    # Evacuate the ctx banks to SBUF right after the final ctx
                    # accumulation (frees the banks for the next qtile fast),
                    # then the reciprocal of the two denominator rows runs off
                    # the PSUM-WAR critical path.
                    dsb = rpool.tile([33, QT], F32, tag="dsb")
                    nc.vector.tensor_copy(dsb[0:1, :], ctxA[64:65, :])
                    nc.vector.tensor_copy(dsb[32:33, :], ctxB[64:65, :])
                    ctxc = rpool.tile([128, QT], F32, tag="ctxc")
                    nc.vector.tensor_copy(ctxc[0:64, :], ctxA[0:64, :])
                    nc.vector.tensor_copy(ctxc[64:128, :], ctxB[0:64, :])
                    rcp = rpool.tile([33, QT], F32, tag="rcp")
                    # one call covers both denominator rows (0 and 32); the
                    # uninitialized rows in between are never read
                    nc.vector.reciprocal(rcp[:, :], dsb[:, :])
                    rsb = rpool.tile([1, 2 * QT], F32, tag="rsb")
                    nc.vector.tensor_copy(rsb[0:1, 0:QT], rcp[0:1, :])
                    nc.vector.tensor_copy(rsb[0:1, QT : 2 * QT], rcp[32:33, :])
                    return rsb, ctxc

                def emit_normalize(qt, rsb, ctxc):
                    # broadcast recips across partitions via rank-1 PE matmuls
                    # into a borrowed score-ring bank, then fused
                    # normalize+cast into ctxn (all SBUF inputs — no ctx-bank
                    # dependence left).
                    qc = slice(qt * QT, (qt + 1) * QT)
                    bt = sgps.tile([128, GRP * QT], F32, tag="sg")
                    nc.tensor.matmul(
                        bt[0:64, 0:QT],
                        lhsT=ones_sb[:],
                        rhs=rsb[0:1, 0:QT],
                        start=True,
                        stop=True,
                    )
                    nc.tensor.matmul(
                        bt[64:128, 0:QT],
                        lhsT=ones_sb[:],
                        rhs=rsb[0:1, QT : 2 * QT],
                        start=True,
                        stop=True,
                        tile_position=(0, 64),
                    )
                    recb = rpool.tile([128, QT], F32, tag="recb")
                    nc.vector.tensor_copy(recb[:], bt[:, 0:QT])
                    nc.vector.tensor_tensor(
                        ctxn[0:64, qc], ctxc[0:64, :], recb[0:64, :], op=mult
                    )
                    nc.vector.tensor_tensor(
                        ctxn[64:128, qc], ctxc[64:128, :], recb[64:128, :], op=mult
                    )

                def emit_outproj(qt, j):
                    # out rows [qt*512 + j*128, +128) = ctxn_chunk^T @ wo
                    ot = sgps.tile([128, GRP * QT], F32, tag="sg")
                    col = qt * QT + j * 128
                    for half in range(2):
                        nc.tensor.matmul(
                            ot[:, half * QT : (half + 1) * QT],
                            lhsT=ctxn[:, col : col + 128],
                            rhs=wo_sb[:, half * QT : (half + 1) * QT],
                            start=True,
                            stop=True,
                        )
                    osb = opool.tile([128, D], F16, tag="osb")
                    nc.vector.tensor_copy(osb[:], ot[:, 0:D])
                    nc.sync.dma_start(
                        out=out_d[col : col + 128, :], in_=osb[:]
                    )

                norm_prev = None
                for qt in range(NQT):
                    pts = {}
                    for g in range(NG):
                        pts[g] = emit_scores(qt, g)
                        if g == 4 and qt > 0:
                            emit_normalize(qt - 1, *norm_prev)
                        if qt > 0 and g in (7, 12, 16, 21):
                            emit_outproj(qt - 1, {7: 0, 12: 1, 16: 2, 21: 3}[g])
                        if g >= 1:
                            emit_ctx(g - 1, pts.pop(g - 1))
                    emit_ctx(NG - 1, pts.pop(NG - 1))
                    norm_prev = emit_recips(qt)
                emit_normalize(NQT - 1, *norm_prev)
                for j in range(4):
                    emit_outproj(NQT - 1, j)

    _split_multi_waits(nc)
    return nc


@functools.cache
def _cached_program() -> bass.Bass:
    return build_program()


def _prep_inputs(hidden_states, freqs_cis, attention_mask, wq, wk, wv, wo):
    hid = np.asarray(hidden_states, np.float32).reshape(S, D)
    hidT = np.ascontiguousarray(hid.T).astype(bf16)

    # within-head channel permutation: evens then odds (rope pairs 32 apart)
    perm1 = np.concatenate([np.arange(0, HD, 2), np.arange(1, HD, 2)])
    perm = np.concatenate([perm1, perm1 + HD])  # for the 2 heads of a core

    fc = np.asarray(freqs_cis, np.float32)
    cosT = np.ascontiguousarray(fc[:, :, 0].T)  # [32, S]
    sinT = np.ascontiguousarray(fc[:, :, 1].T)
    cosf = np.concatenate([cosT, cosT, cosT, cosT], 0).astype(bf16)
    sinf = np.concatenate([-sinT, sinT, -sinT, sinT], 0).astype(bf16)

    mask_add = (1.0 - np.asarray(attention_mask, np.float32).reshape(S)) * -10000.0
    maskadd = np.ascontiguousarray(mask_add.reshape(NKC, 128).T).astype(np.float32)

    def wlayout(w):  # [1024, 128] -> [128 partitions, chunk-major 1024]
        w = np.ascontiguousarray(w)
        return np.ascontiguousarray(
            w.reshape(NDC, 128, 128).transpose(1, 0, 2).reshape(128, D)
        ).astype(bf16)

    in_maps = []
    for core in range(8):
        cols = slice(core * 128, (core + 1) * 128)
        in_maps.append(
            {
                "hidT": hidT,
                "wq": wlayout(np.asarray(wq, np.float32)[:, cols][:, perm]),
                "wk": wlayout(np.asarray(wk, np.float32)[:, cols][:, perm]),
                "wv": wlayout(np.asarray(wv, np.float32)[:, cols]),
                "wo": np.ascontiguousarray(np.asarray(wo, np.float32)[cols, :]).astype(bf16),
                "cosf": cosf,
                "sinf": sinf,
                "maskadd": maskadd,
                "ones64": np.ones((1, 64), np.float32),
            }
        )
    return in_maps


def run_sharded(in_maps, **kwargs):
    nc = _cached_program()
    return run_bass_kernel_spmd(nc, in_maps, list(range(8)), **kwargs)


def kernel(hidden_states, freqs_cis, attention_mask, wq, wk, wv, wo):
    in_maps = _prep_inputs(
        hidden_states, freqs_cis, attention_mask, wq, wk, wv, wo
    )
    res = run_sharded(in_maps).results
    out = np.zeros((S, D), np.float32)
    for r in res:
        out += np.asarray(r["outp"], np.float32)
    return out.reshape(1, S, D)


if __name__ == "__main__":
    import reference

    inputs = reference.setup_inputs()
    inputs = {k: np.asarray(v) for k, v in inputs.items()}
    expected = np.asarray(reference.reference(**inputs))
    actual = kernel(**inputs)
    err = np.abs(actual - expected).max() / np.abs(expected).max()
    print("Relative error:", err)


# revision 19
# speedup vs baseline: 1.8262x; 1.0249x over previous
"""Trainium2 Bass kernel for 16-head MHA with RoPE (B=1, S=4096, D=1024).

Sharding: tensor-parallel over heads — 2 heads per core on 8 cores.
Per-core pipeline (all matmuls bf16, fp32 PSUM accumulation):
  1. hidT [d, s] (host-prepped bf16) streamed over BOTH hwdge DMA queues;
     weights/cos/sin ride the scalar queue behind the odd hidT chunks.
  2. Projections Q, V, K run through the SAME two 3-bank PSUM ring tiles
     the attention scores use (tag-shared pool) — the ctx banks are never
     touched by projections, so attention starts with zero PSUM handoff
     stalls. Q streams 6 strips DMA-paced + 2 strips in a second pass;
     V/K are strip-ordered with PSUM casts on ScalarE.
  3. RoPE in bf16 via the partition-swap trick (channels permuted
     host-side to [evens | odds] per head). Q full-width; K per 512-col
     segment right behind its cast so scores chase the rope segments.
  4. V transposed (xbar DMA) to v_nat [s, c], then one fused
     f-scale+restride pass into 66-col blocks per (chunk, head) with f
     appended as a 65th stationary column — the ctx matmul accumulates the
     softmax denominator into PSUM partition 64 for free.
  5. Attention per q-tile of 512, transposed scores S_T[k, q]: exp on
     ScalarE (scale=1/8 folded), ctx+den accumulate over 32 k-chunks into
     two [65, 512] PSUM banks (one per head). Scores double-buffered with
     one-group lookahead so the PE never stalls on the exp.
  6. Denominators + ctx evacuated to SBUF right after the last ctx matmul
     (frees the banks), then reciprocal, rank-1 PE broadcast through a
     borrowed ring bank, and fused normalize+cast run off the WAR path.
  7. Out-projection of q-tile t interleaved into q-tile t+1 through the
     same PSUM ring; partials written to DRAM as fp16.
Host sums the 8 fp16 partials in fp32.
"""

import functools

import numpy as np
import ml_dtypes

import concourse.bass as bass
import concourse.tile as tile
import concourse.mybir as mybir
from concourse.bass_utils import run_bass_kernel_spmd

BF16 = mybir.dt.bfloat16
F16 = mybir.dt.float16
F32 = mybir.dt.float32
bf16 = ml_dtypes.bfloat16

S = 4096      # sequence length
D = 1024      # model dim
HD = 64       # head dim
C = 128       # channels per core (2 heads)
NDC = 8       # contraction chunks of 128 over D
NKC = 32      # key chunks of 128 over S
NQT = 8       # query tiles of 512
QT = 512
GRP = 3       # score (chunk, head) slots per exp group / PSUM bank triple
VB = 66       # v4 block stride per (chunk, head): 64 V cols + f col + pad


_NO_SPLIT = (
    mybir.InstEventSemaphore,
    mybir.InstUnconditionalBranch,
)


def _split_multi_waits(nc: bass.Bass) -> None:
    """Hoist extra sem waits onto standalone EventSemaphore carriers.

    This walrus build only supports one sync-wait command per engine
    instruction ("Too many sync wait commands" in setupSyncWait), so any
    instruction Tile scheduled with >1 wait gets all but its last wait moved
    to dedicated InstEventSemaphore instructions placed immediately before it
    in the same engine stream (sequencer blocks on them in program order —
    semantically identical).
    """
    n = 0
    for fn in nc.m.functions:
        for blk in fn.blocks:
            out = []
            for inst in blk.instructions:
                si = inst.sync_info
                if (
                    si is not None
                    and si.on_wait
                    and len(si.on_wait) > 1
                    and not isinstance(inst, _NO_SPLIT)
                    and inst.engine != mybir.EngineType.Unassigned
                ):
                    waits = list(si.on_wait)
                    for w in waits[:-1]:
                        ev = mybir.InstEventSemaphore(name=f"ant_waitsplit_{n}")
                        n += 1
                        ev.engine = inst.engine
                        ev.sync_info = mybir.SyncInfo(on_wait=[w], on_update=[])
                        nc.register_instruction(ev)
                        out.append(ev)
                    si.on_wait = [waits[-1]]
                    inst.sync_info = si
                out.append(inst)
            blk.instructions[:] = out


def build_program() -> bass.Bass:
    nc = bass.Bass()
    hidT_d = nc.declare_dram_parameter("hidT", [D, S], BF16, isOutput=False)
    wq_d = nc.declare_dram_parameter("wq", [128, D], BF16, isOutput=False)
    wk_d = nc.declare_dram_parameter("wk", [128, D], BF16, isOutput=False)
    wv_d = nc.declare_dram_parameter("wv", [128, D], BF16, isOutput=False)
    wo_d = nc.declare_dram_parameter("wo", [128, D], BF16, isOutput=False)
    cos_d = nc.declare_dram_parameter("cosf", [128, S], BF16, isOutput=False)
    sin_d = nc.declare_dram_parameter("sinf", [128, S], BF16, isOutput=False)
    mask_d = nc.declare_dram_parameter("maskadd", [128, NKC], F32, isOutput=False)
    ones_d = nc.declare_dram_parameter("ones64", [1, 64], F32, isOutput=False)
    out_d = nc.declare_dram_parameter("outp", [S, D], F16, isOutput=True)

    Exp = mybir.ActivationFunctionType.Exp
    mult = mybir.AluOpType.mult
    add = mybir.AluOpType.add

    with tile.TileContext(nc) as tc:
        with (
            tc.tile_pool(name="const", bufs=1) as const,
            tc.tile_pool(name="ppool", bufs=3) as ppool,
            tc.tile_pool(name="sgps", bufs=2, space="PSUM") as sgps,
            tc.tile_pool(name="ctxps", bufs=1, space="PSUM") as ctxps,
            tc.tile_pool(name="rpool", bufs=2) as rpool,
            tc.tile_pool(name="opool", bufs=3) as opool,
        ):
            # ---- persistent SBUF tiles -------------------------------------
            wq_sb = const.tile([128, D], BF16, tag="wq")
            wk_sb = const.tile([128, D], BF16, tag="wk")
            wv_sb = const.tile([128, D], BF16, tag="wv")
            wo_sb = const.tile([128, D], BF16, tag="wo")
            mask_sb = const.tile([128, NKC], F32, tag="mask")
            f_f32 = const.tile([128, NKC], F32, tag="ff32")
            ones_sb = const.tile([1, 64], F32, tag="ones")
            cos_sb = const.tile([128, S], BF16, tag="cosf")
            sin_sb = const.tile([128, S], BF16, tag="sinf")
            qT_bf = const.tile([128, S], BF16, tag="qTbf")
            kT_bf = const.tile([128, S], BF16, tag="kTbf")
            qsw = const.tile([128, S], BF16, tag="qsw")
            v4 = const.tile([128, NKC * 2 * VB], BF16, tag="v4")
            ctxn = const.tile([128, S], BF16, tag="ctxn")
            hidT_sb = const.tile([128, NDC * S], BF16, tag="hidT")
            vT_bf = const.tile([128, S], BF16, tag="vTbf")
            v_nat = const.tile([128, S], BF16, tag="vnat")

            # hidT alternates between the two hwdge queues; the scalar queue
            # additionally carries everything that is needed later, ordered
            # by first use (wv ~t25, cos/sin ~t25, wk ~t30, wo ~t90).
            nc.sync.dma_start(out=wq_sb[:], in_=wq_d[:])
            nc.scalar.dma_start(out=mask_sb[:], in_=mask_d[:])
            for dc in range(NDC):
                eng = nc.sync if dc % 2 == 0 else nc.scalar
                eng.dma_start(
                    out=hidT_sb[:, dc * S : (dc + 1) * S],
                    in_=hidT_d[dc * 128 : (dc + 1) * 128, :],
                )
            nc.scalar.dma_start(out=wv_sb[:], in_=wv_d[:])
            nc.sync.dma_start(out=cos_sb[:], in_=cos_d[:])
            nc.sync.dma_start(out=sin_sb[:], in_=sin_d[:])
            nc.scalar.dma_start(out=wk_sb[:], in_=wk_d[:])
            nc.scalar.dma_start(out=ones_sb[:], in_=ones_d[:])
            nc.scalar.dma_start(out=wo_sb[:], in_=wo_d[:])
            # f[k] = exp(mask_add[k]) — also warms the ACT exp table early
            nc.scalar.activation(f_f32[:], mask_sb[:], Exp)

            # ---- PSUM layout ----------------------------------------------
            ctxA = ctxps.tile([65, QT], F32, tag="ctxA")
            ctxB = ctxps.tile([65, QT], F32, tag="ctxB")
            ctx_banks = (ctxA, ctxB)

            # pre-fill both dsb ring buffers so the single batched
            # reciprocal's untouched rows (1..31) stay finite
            for _ in range(2):
                dpre = rpool.tile([33, QT], F32, tag="dsb", name="dpre")
                nc.vector.memset(dpre[:], 1.0)

            # ---- phase 1: projections through the score ring ---------------
            def mm_proj(t, j, w_sb, strip, dc):
                nc.tensor.matmul(
                    t[:, j * QT : (j + 1) * QT],
                    lhsT=w_sb[:, dc * 128 : (dc + 1) * 128],
                    rhs=hidT_sb[:, dc * S + strip * QT : dc * S + (strip + 1) * QT],
                    start=(dc == 0),
                    stop=(dc == NDC - 1),
                )

            def proj_ring(w_sb, dst, dma_paced=False, cast3=None, per_strip=None):
                """8 strips of 512 cols via ring tiles of 3+3+2 strips."""
                if cast3 is None:
                    cast3 = nc.scalar.copy
                t0 = sgps.tile([128, GRP * QT], F32, tag="sg", name="pj0")
                t1 = sgps.tile([128, GRP * QT], F32, tag="sg", name="pj1")
                if dma_paced:
                    # strips 0-5 consume hidT chunks as they arrive; strips
                    # 6-7 re-read SBUF-resident chunks in a second pass
                    for dc in range(NDC):
                        for strip in range(6):
                            mm_proj(t0 if strip < 3 else t1, strip % 3, w_sb, strip, dc)
                else:
                    for strip in range(3):
                        for dc in range(NDC):
                            mm_proj(t0, strip, w_sb, strip, dc)
                    for strip in range(3, 6):
                        for dc in range(NDC):
                            mm_proj(t1, strip - 3, w_sb, strip, dc)
                cast3(dst[:, 0 : 3 * QT], t0[:, :])
                if per_strip is not None:
                    for st in range(3):
                        per_strip(st)
                t2 = sgps.tile([128, GRP * QT], F32, tag="sg", name="pj2")
                for strip in range(6, 8):
                    for dc in range(NDC):
                        mm_proj(t2, strip - 6, w_sb, strip, dc)
                cast3(dst[:, 3 * QT : 6 * QT], t1[:, :])
                if per_strip is not None:
                    for st in range(3, 6):
                        per_strip(st)
                cast3(dst[:, 6 * QT : 8 * QT], t2[:, 0 : 2 * QT])
                if per_strip is not None:
                    for st in range(6, 8):
                        per_strip(st)

            def rope(x_bf, s0, s1):
                # channel rows per head h: [h*64, h*64+32) = evens ("a"),
                # [h*64+32, h*64+64) = odds ("b");
                # out = x * cos_full + swap(x) * sin_signed
                sc = slice(s0, s1)
                for h in range(2):
                    a = slice(h * 64, h * 64 + 32)
                    b = slice(h * 64 + 32, h * 64 + 64)
                    nc.vector.tensor_copy(qsw[a, sc], x_bf[b, sc])
                    nc.vector.tensor_copy(qsw[b, sc], x_bf[a, sc])
                nc.vector.tensor_tensor(x_bf[:, sc], x_bf[:, sc], cos_sb[:, sc], op=mult)
                nc.vector.tensor_tensor(qsw[:, sc], qsw[:, sc], sin_sb[:, sc], op=mult)
                nc.vector.tensor_tensor(x_bf[:, sc], x_bf[:, sc], qsw[:, sc], op=add)

            v4r = v4[:].rearrange("p (kc h c) -> p kc h c", kc=NKC, h=2)
            vnr = v_nat[:].rearrange("p (kc h c) -> p kc h c", kc=NKC, h=2)

            def restride(kc0, kc1, eng_scalar):
                # fused f-scale + restride of v_nat chunks into v4 blocks
                for kc in range(kc0, kc1):
                    if eng_scalar:
                        nc.scalar.activation(
                            v4r[:, kc : kc + 1, :, 0:64],
                            vnr[:, kc : kc + 1, :, :],
                            mybir.ActivationFunctionType.Copy,
                            scale=f_f32[:, kc : kc + 1],
                        )
                    else:
                        nc.vector.tensor_scalar(
                            v4r[:, kc : kc + 1, :, 0:64],
                            vnr[:, kc : kc + 1, :, :],
                            f_f32[:, kc : kc + 1],
                            None,
                            op0=mult,
                        )

            # Q: DMA-paced projection, then full-width rope on the DVE
            proj_ring(wq_sb, qT_bf, dma_paced=True)
            rope(qT_bf, 0, S)

            # V: projection + transpose; restride happens on the DVE between
            # the q-rope and the k-rope chain (first half) and after it
            # (second half)
            proj_ring(wv_sb, vT_bf)
            nc.sync.dma_start_transpose(
                out=v_nat[:].rearrange("p (kc c) -> p kc c", kc=NKC),
                in_=vT_bf[:],
            )
            restride(0, 16, eng_scalar=False)
            for h in range(2):
                nc.scalar.copy(
                    v4r[:, :, h : h + 1, 64:65],
                    f_f32[:].unsqueeze(-1).unsqueeze(-1),
                )

            # K: projection with per-512-segment rope chasing the casts
            proj_ring(
                wk_sb,
                kT_bf,
                per_strip=lambda st: rope(kT_bf, st * QT, (st + 1) * QT),
            )
            restride(16, NKC, eng_scalar=False)

            # ---- phase 2: attention + fused out-projection -----------------
            slots = [(c, h) for c in range(NKC) for h in range(2)]
            groups = [slots[i : i + GRP] for i in range(0, len(slots), GRP)]
            NG = len(groups)  # 22

            def emit_scores(qt, g):
                qc = slice(qt * QT, (qt + 1) * QT)
                grp = groups[g]
                nb = len(grp)
                sg = sgps.tile([128, GRP * QT], F32, tag="sg")
                Pt = ppool.tile([128, GRP * QT], BF16, tag="pt")
                for i, (c, h) in enumerate(grp):
                    hr = slice(h * 64, (h + 1) * 64)
                    nc.tensor.matmul(
                        sg[:, i * QT : (i + 1) * QT],
                        lhsT=kT_bf[hr, c * 128 : (c + 1) * 128],
                        rhs=qT_bf[hr, qc],
                        start=True,
                        stop=True,
                    )
                nc.scalar.activation(
                    Pt[:, : nb * QT], sg[:, : nb * QT], Exp, scale=0.125
                )
                return Pt

            def emit_ctx(g, Pt):
                grp = groups[g]
                for i, (c, h) in enumerate(grp):
                    vcol = (c * 2 + h) * VB
                    nc.tensor.matmul(
                        ctx_banks[h][:, :],
                        lhsT=v4[:, vcol : vcol + 65],
                        rhs=Pt[:, i * QT : (i + 1) * QT],
                        start=(c == 0),
                        stop=(c == NKC - 1),
                    )

            def emit_recips(qt):
                # Evacuate the ctx banks to SBUF right after the final ctx
                # accumulation (frees the banks for the next qtile fast),
                # then the reciprocal of the two denominator rows runs off
                # the PSUM-WAR critical path.
                dsb = rpool.tile([33, QT], F32, tag="dsb")
                nc.vector.tensor_copy(dsb[0:1, :], ctxA[64:65, :])
                nc.vector.tensor_copy(dsb[32:33, :], ctxB[64:65, :])
                ctxc = rpool.tile([128, QT], F32, tag="ctxc")
                nc.vector.tensor_copy(ctxc[0:64, :], ctxA[0:64, :])
                nc.vector.tensor_copy(ctxc[64:128, :], ctxB[0:64, :])
                rcp = rpool.tile([33, QT], F32, tag="rcp")
                # one call covers both denominator rows (0 and 32); the
                # pre-memset rows in between stay finite and unread
                nc.vector.reciprocal(rcp[:, :], dsb[:, :])
                rsb = rpool.tile([1, 2 * QT], F32, tag="rsb")
                nc.vector.tensor_copy(rsb[0:1, 0:QT], rcp[0:1, :])
                nc.vector.tensor_copy(rsb[0:1, QT : 2 * QT], rcp[32:33, :])
                return rsb, ctxc

            def emit_normalize(qt, rsb, ctxc):
                # broadcast recips across partitions via rank-1 PE matmuls
                # into a borrowed score-ring bank, then fused normalize+cast
                # into ctxn (all SBUF inputs — no ctx-bank dependence left).
                qc = slice(qt * QT, (qt + 1) * QT)
                bt = sgps.tile([128, GRP * QT], F32, tag="sg")
                nc.tensor.matmul(
                    bt[0:64, 0:QT],
                    lhsT=ones_sb[:],
                    rhs=rsb[0:1, 0:QT],
                    start=True,
                    stop=True,
                )
                nc.tensor.matmul(
                    bt[64:128, 0:QT],
                    lhsT=ones_sb[:],
                    rhs=rsb[0:1, QT : 2 * QT],
                    start=True,
                    stop=True,
                    tile_position=(0, 64),
                )
                recb = rpool.tile([128, QT], F32, tag="recb")
                nc.vector.tensor_copy(recb[:], bt[:, 0:QT])
                nc.vector.tensor_tensor(
                    ctxn[0:64, qc], ctxc[0:64, :], recb[0:64, :], op=mult
                )
                nc.vector.tensor_tensor(
                    ctxn[64:128, qc], ctxc[64:128, :], recb[64:128, :], op=mult
                )

            def emit_outproj(qt, j, cast_eng=None):
                # out rows [qt*512 + j*128, +128) = ctxn_chunk^T @ wo
                ot = sgps.tile([128, GRP * QT], F32, tag="sg")
                col = qt * QT + j * 128
                for half in range(2):
                    nc.tensor.matmul(
                        ot[:, half * QT : (half + 1) * QT],
                        lhsT=ctxn[:, col : col + 128],
                        rhs=wo_sb[:, half * QT : (half + 1) * QT],
                        start=True,
                        stop=True,
                    )
                osb = opool.tile([128, D], F16, tag="osb")
                (cast_eng or nc.vector.tensor_copy)(osb[:], ot[:, 0:D])
                nc.sync.dma_start(
                    out=out_d[col : col + 128, :], in_=osb[:]
                )

            norm_prev = None
            for qt in range(NQT):
                pts = {}
                for g in range(NG):
                    pts[g] = emit_scores(qt, g)
                    if g == 4 and qt > 0:
                        emit_normalize(qt - 1, *norm_prev)
                    if qt > 0 and g in (7, 12, 16, 21):
                        emit_outproj(qt - 1, {7: 0, 12: 1, 16: 2, 21: 3}[g])
                    if g >= 1:
                        emit_ctx(g - 1, pts.pop(g - 1))
                emit_ctx(NG - 1, pts.pop(NG - 1))
                norm_prev = emit_recips(qt)
            emit_normalize(NQT - 1, *norm_prev)
            for j in range(4):
                # alternate the final copies between DVE and ScalarE to
                # shorten the serial tail
                emit_outproj(
                    NQT - 1, j,
                    cast_eng=nc.scalar.copy if j % 2 else nc.vector.tensor_copy,
                )

    _split_multi_waits(nc)
    return nc


@functools.cache
def _cached_program() -> bass.Bass:
    return build_program()


def _prep_inputs(hidden_states, freqs_cis, attention_mask, wq, wk, wv, wo):
    hid = np.asarray(hidden_states, np.float32).reshape(S, D)
    hidT = np.ascontiguousarray(hid.T).astype(bf16)

    # within-head channel permutation: evens then odds (rope pairs 32 apart)
    perm1 = np.concatenate([np.arange(0, HD, 2), np.arange(1, HD, 2)])
    perm = np.concatenate([perm1, perm1 + HD])  # for the 2 heads of a core

    fc = np.asarray(freqs_cis, np.float32)
    cosT = np.ascontiguousarray(fc[:, :, 0].T)  # [32, S]
    sinT = np.ascontiguousarray(fc[:, :, 1].T)
    cosf = np.concatenate([cosT, cosT, cosT, cosT], 0).astype(bf16)
    sinf = np.concatenate([-sinT, sinT, -sinT, sinT], 0).astype(bf16)

    mask_add = (1.0 - np.asarray(attention_mask, np.float32).reshape(S)) * -10000.0
    maskadd = np.ascontiguousarray(mask_add.reshape(NKC, 128).T).astype(np.float32)

    def wlayout(w):  # [1024, 128] -> [128 partitions, chunk-major 1024]
        w = np.ascontiguousarray(w)
        return np.ascontiguousarray(
            w.reshape(NDC, 128, 128).transpose(1, 0, 2).reshape(128, D)
        ).astype(bf16)

    in_maps = []
    for core in range(8):
        cols = slice(core * 128, (core + 1) * 128)
        in_maps.append(
            {
                "hidT": hidT,
                "wq": wlayout(np.asarray(wq, np.float32)[:, cols][:, perm]),
                "wk": wlayout(np.asarray(wk, np.float32)[:, cols][:, perm]),
                "wv": wlayout(np.asarray(wv, np.float32)[:, cols]),
                "wo": np.ascontiguousarray(np.asarray(wo, np.float32)[cols, :]).astype(bf16),
                "cosf": cosf,
                "sinf": sinf,
                "maskadd": maskadd,
                "ones64": np.ones((1, 64), np.float32),
            }
        )
    return in_maps


def run_sharded(in_maps, **kwargs):
    nc = _cached_program()
    return run_bass_kernel_spmd(nc, in_maps, list(range(8)), **kwargs)


def kernel(hidden_states, freqs_cis, attention_mask, wq, wk, wv, wo):
    in_maps = _prep_inputs(
        hidden_states, freqs_cis, attention_mask, wq, wk, wv, wo
    )
    res = run_sharded(in_maps).results
    out = np.zeros((S, D), np.float32)
    for r in res:
        out += np.asarray(r["outp"], np.float32)
    return out.reshape(1, S, D)


if __name__ == "__main__":
    import reference

    inputs = reference.setup_inputs()
    inputs = {k: np.asarray(v) for k, v in inputs.items()}
    expected = np.asarray(reference.reference(**inputs))
    actual = kernel(**inputs)
    err = np.abs(actual - expected).max() / np.abs(expected).max()
    print("Relative error:", err)
